# revision 14
# baseline (speedup 1.0000x reference)
"""Trainium2 Bass kernel for nn_MinCutExplainerGNN (8 NeuronCores, SPMD).

Structure (3 SPMD launches, no collectives):
  P1 (x-phase, nodes sharded 8-way): S = softmax(x@Wa+ba); partials
     ZX = S^T x, SSaug = S^T [S|1], vol = sum(deg * rowsum(S)); S fp32 out +
     bf16 gather table shard.
  host: concatenates the bf16 S table (pure data movement), edge index
     preprocessing (integer only: bucketing by destination shard, class split
     by (pos_r&3, pos_c&3) for 256B-line gather addressing, padding).
  P2 (edge phase, edges sharded by destination): two-sided dma_gather of S
     rows (64B bf16 payloads out of 256B-stride lines), adj partial
     accumulation via PSUM outer-product matmuls.
  P3 (replicated finisher): reduce partials, Z = ZX@Wp + colsum*bp, losses,
     the two dense-K explainer layers, final linear.
"""
import numpy as np
import ml_dtypes
import jax
from jax.sharding import Mesh, PartitionSpec
from jax.experimental.shard_map import shard_map

import concourse.bacc as bacc
import concourse.bass as bass
import concourse.mybir as mybir
import concourse.tile as tile
from concourse import ap_utils
from concourse.bass import MemorySpace
from concourse._compat import exact_div
from concourse import bass2jax
from concourse.bass2jax import _bass_exec_p, partition_id_tensor
from concourse.masks import make_identity

P = 128
NCORE = 8
N = 100000
D = 512
K = 30
EPS = 1e-9
SH_REAL = 12500            # real rows per shard
SH = 12544                 # padded rows per shard (98 * 128)
TILES = SH // P            # 98
NPOS = NCORE * SH          # 100352
NLINES = NPOS // 4 + 128   # table lines (4 rows/256B line) + zero pad lines
F32, F32R, BF16, I16 = (mybir.dt.float32, mybir.dt.float32r,
                        mybir.dt.bfloat16, mybir.dt.int16)
AX = mybir.AxisListType.X
CHUNK = 12800              # idxs per dma_gather instruction


# ---------------------------------------------------------------- utilities

def dma_gather_raw(gp, out_ap, in_ap, idxs_ap, num_idxs, elem_size, elem_step,
                   queue_num=0):
    gp._assert_queue_num(queue_num)
    assert idxs_ap.dtype == I16
    assert in_ap.space == MemorySpace.DRAM
    assert in_ap.dtype == out_ap.dtype
    assert num_idxs % 128 == 0
    assert ap_utils.ap_is_contiguous(out_ap.ap[1:])
    assert ap_utils.ap_is_contiguous(idxs_ap.ap[1:])
    assert in_ap.ap[-1][1] == out_ap.ap[-1][1] == elem_size
    assert out_ap.ap[0][1] * out_ap.ap[1][1] == num_idxs
    assert in_ap.ap[0][0] == elem_step
    stride_bytes_256 = exact_div(elem_step * mybir.dt.size(in_ap.dtype), 256)
    _in_ap = gp.lower_ap_dma(in_ap, for_custom_bir_dma=True)
    return gp.add_instruction(
        mybir.InstDMAGatherAnt(
            name=gp.bass.get_next_instruction_name(),
            ins=[*_in_ap, gp.lower_ap(idxs_ap),
                 gp.lower_val_access(gp.to_reg(num_idxs))],
            outs=[gp.lower_ap(out_ap)],
            transpose=False, num_idxs=num_idxs, elem_size=elem_size,
            stride_bytes_256=stride_bytes_256, gen_mode=0, single_packet=False,
            queue_num=queue_num, sbuf_tokens_per_rank=0,
            sbuf_free_dim_per_rank=0, sbuf_free_dim_pad_per_rank=0,
            sbuf_byte_offset=0,
        ))


def wrap_idxs(idx):
    n = idx.shape[0]
    w = idx.astype(np.int16).reshape(n // 16, 16).T
    return np.tile(w, (8, 1))


class SpmdRunner:
    def __init__(self, nc, n_cores=NCORE):
        if not nc.is_finalized():
            nc.finalize()
        self.nc, self.n_cores = nc, n_cores
        bass2jax.install_neuronx_cc_hook()
        pname = nc.partition_id_tensor.name if nc.partition_id_tensor else None
        in_names, out_names, out_avals, zouts = [], [], [], []
        for alloc in nc.m.functions[0].allocations:
            if not isinstance(alloc, mybir.MemoryLocationSet):
                continue
            name = alloc.memorylocations[0].name
            if alloc.kind == "ExternalInput":
                if name != pname:
                    in_names.append(name)
            elif alloc.kind == "ExternalOutput":
                shape, dtype = tuple(alloc.tensor_shape), mybir.dt.np(alloc.dtype)
                out_names.append(name)
                out_avals.append(jax.core.ShapedArray(shape, dtype))
                zouts.append((shape, dtype))
        self.in_names, self.out_names = in_names, out_names
        self.out_avals, self.zero_outs = out_avals, zouts
        n_params, n_outs = len(in_names), len(out_avals)
        all_in = list(in_names) + list(out_names) + ([pname] if pname else [])

        def _body(*args):
            operands = list(args)
            if pname is not None:
                operands.append(partition_id_tensor())
            return tuple(_bass_exec_p.bind(
                *operands, out_avals=tuple(out_avals), in_names=tuple(all_in),
                out_names=tuple(out_names), lowering_input_output_aliases=(),
                sim_require_finite=True, sim_require_nnan=True, nc=nc))

        devices = jax.devices()[:n_cores]
        mesh = Mesh(np.asarray(devices), ("core",))
        self.sharded = jax.jit(
            shard_map(_body, mesh=mesh,
                      in_specs=(PartitionSpec("core"),) * (n_params + n_outs),
                      out_specs=(PartitionSpec("core"),) * n_outs,
                      check_rep=False),
            donate_argnums=tuple(range(n_params, n_params + n_outs)),
            keep_unused=True)

    def __call__(self, in_maps):
        concat_in = [np.concatenate([np.asarray(in_maps[c][k])
                                     for c in range(self.n_cores)], axis=0)
                     for k in self.in_names]
        zeros = [np.zeros((self.n_cores * s[0], *s[1:]), d)
                 for (s, d) in self.zero_outs]
        outs = self.sharded(*concat_in, *zeros)
        jax.block_until_ready(outs)
        return [{k: np.asarray(outs[i]).reshape(self.n_cores,
                                                *self.out_avals[i].shape)[c]
                 for i, k in enumerate(self.out_names)}
                for c in range(self.n_cores)]


# ------------------------------------------------------------------ P1 build

def build_p1():
    nc = bacc.Bacc(None, num_devices=NCORE)
    x_t = nc.dram_tensor("x", [SH, D], F32, kind="ExternalInput")
    wa_t = nc.dram_tensor("wa", [D, K], F32, kind="ExternalInput")
    ba_t = nc.dram_tensor("ba", [1, K], F32, kind="ExternalInput")
    deg_t = nc.dram_tensor("deg", [SH, 1], F32, kind="ExternalInput")
    s_out = nc.dram_tensor("s_out", [TILES, P, K], F32, kind="ExternalOutput")
    sb_out = nc.dram_tensor("sb_out", [TILES, P, 32], BF16, kind="ExternalOutput")
    zx_out = nc.dram_tensor("zx_out", [K, D], F32, kind="ExternalOutput")
    ss_out = nc.dram_tensor("ss_out", [K, K + 2], F32, kind="ExternalOutput")
    vol_out = nc.dram_tensor("vol_out", [1, 1], F32, kind="ExternalOutput")

    with nc.allow_low_precision(reason="float32r operands"), \
         tile.TileContext(nc) as tc:
        with (tc.tile_pool(name="consts", bufs=1) as cp,
              tc.tile_pool(name="sbuf", bufs=3) as sp,
              tc.tile_pool(name="psum", bufs=2, space="PSUM") as pp,
              tc.tile_pool(name="psacc", bufs=1, space="PSUM") as pa):
            ident0 = cp.tile([P, P], F32)
            make_identity(nc, ident0[:])
            ident = cp.tile([P, P], F32R)
            nc.vector.tensor_copy(out=ident[:], in_=ident0[:])
            wa0 = cp.tile([P, 4, K], F32)
            nc.sync.dma_start(out=wa0[:], in_=wa_t.rearrange("(c p) k -> p c k", p=P))
            wa = cp.tile([P, 4, K], F32R)
            nc.vector.tensor_copy(out=wa[:], in_=wa0[:])
            ones1f = cp.tile([1, P], F32)
            nc.vector.memset(ones1f[:], 1.0)
            ones1 = cp.tile([1, P], F32R)
            nc.vector.tensor_copy(out=ones1[:], in_=ones1f[:])
            ba0 = cp.tile([1, K], F32)
            nc.sync.dma_start(out=ba0[:], in_=ba_t[:, :])
            ba = cp.tile([1, K], F32R)
            nc.vector.tensor_copy(out=ba[:], in_=ba0[:])
            vol_acc = cp.tile([P, 1], F32)
            nc.vector.memset(vol_acc[:], 0.0)
            ones_col = cp.tile([P, 1], F32)
            nc.vector.memset(ones_col[:], 1.0)
            zero_col = cp.tile([P, 1], F32)
            nc.vector.memset(zero_col[:], 0.0)
            valid = cp.tile([P, 1], F32)
            nc.vector.memset(valid[:], 1.0)
            nc.gpsimd.affine_select(out=valid[:], in_=valid[:],
                                    compare_op=mybir.AluOpType.is_ge,
                                    fill=0.0, base=SH_REAL - 97 * P - 1,
                                    pattern=[[0, 1]], channel_multiplier=-1)
            ps_zx = pa.tile([K, D], F32, tag="zx")
            ps_ss = pa.tile([K, K + 2], F32, tag="ss")

            for t in range(TILES):
                xt0 = sp.tile([P, D], F32, tag="x0")
                nc.sync.dma_start(out=xt0[:], in_=x_t[t * P:(t + 1) * P, :])
                xt = sp.tile([P, D], F32R, tag="x")
                nc.vector.tensor_copy(out=xt[:], in_=xt0[:])
                xT = sp.tile([P, 4, P], F32R, tag="xT")
                for c in range(4):
                    tp = pp.tile([P, P], F32R, tag="tp")
                    nc.tensor.transpose(out=tp[:], in_=xt[:, c * P:(c + 1) * P],
                                        identity=ident[:])
                    nc.vector.tensor_copy(out=xT[:, c, :], in_=tp[:])
                # logits [P, K] = x @ Wa + ba
                lg = pp.tile([P, K], F32, tag="lg")
                for c in range(4):
                    nc.tensor.matmul(out=lg[:], lhsT=xT[:, c, :], rhs=wa[:, c, :],
                                     start=(c == 0), stop=False)
                nc.tensor.matmul(out=lg[:], lhsT=ones1[:], rhs=ba[:],
                                 start=False, stop=True)
                ex = sp.tile([P, K], F32, tag="ex")
                nc.scalar.activation(out=ex[:], in_=lg[:],
                                     func=mybir.ActivationFunctionType.Exp)
                ssum = sp.tile([P, 1], F32, tag="ssum")
                nc.vector.reduce_sum(out=ssum[:], in_=ex[:], axis=AX)
                rcp = sp.tile([P, 1], F32, tag="rcp")
                nc.vector.reciprocal(out=rcp[:], in_=ssum[:])
                s_f = sp.tile([P, K], F32R, tag="sf")
                nc.vector.tensor_scalar_mul(out=s_f[:], in0=ex[:], scalar1=rcp[:])
                if t == TILES - 1:
                    # zero padding rows 12500..12543 (partitions 84..127)
                    nc.vector.tensor_scalar_mul(out=s_f[:], in0=s_f[:].bitcast(F32),
                                                scalar1=valid[:])
                # outputs: S fp32 + bf16 table shard
                nc.sync.dma_start(out=s_out[t, :, :], in_=s_f[:].bitcast(F32))
                s_b = sp.tile([P, 32], BF16, tag="sb")
                nc.vector.memset(s_b[:, K:], 0.0)
                nc.vector.tensor_copy(out=s_b[:, :K], in_=s_f[:])
                nc.sync.dma_start(out=sb_out[t, :, :], in_=s_b[:])
                # ZX += S^T x ; SSaug += S^T [S|1]
                nc.tensor.matmul(out=ps_zx[:], lhsT=s_f[:], rhs=xt[:],
                                 start=(t == 0), stop=(t == TILES - 1))
                s_aug = sp.tile([P, K + 2], F32R, tag="saug")
                nc.vector.tensor_copy(out=s_aug[:, :K], in_=s_f[:])
                nc.vector.tensor_copy(out=s_aug[:, K + 1:], in_=zero_col[:])
                if t == TILES - 1:
                    nc.vector.tensor_copy(out=s_aug[:, K:K + 1], in_=valid[:])
                else:
                    nc.vector.tensor_copy(out=s_aug[:, K:K + 1], in_=ones_col[:])
                nc.tensor.matmul(out=ps_ss[:], lhsT=s_f[:], rhs=s_aug[:],
                                 start=(t == 0), stop=(t == TILES - 1))
                # vol partial: sum(deg * rowsum(S))
                dg = sp.tile([P, 1], F32, tag="dg")
                nc.sync.dma_start(out=dg[:], in_=deg_t[t * P:(t + 1) * P, :])
                rs = sp.tile([P, 1], F32, tag="rs")
                nc.vector.reduce_sum(out=rs[:], in_=s_f[:].bitcast(F32), axis=AX)
                dv = sp.tile([P, 1], F32, tag="dv")
                nc.vector.tensor_mul(out=dv[:], in0=rs[:], in1=dg[:])
                nc.vector.tensor_add(out=vol_acc[:], in0=vol_acc[:], in1=dv[:])

            zx_sb = sp.tile([K, D], F32)
            nc.vector.tensor_copy(out=zx_sb[:], in_=ps_zx[:])
            nc.sync.dma_start(out=zx_out[:, :], in_=zx_sb[:])
            ss_sb = sp.tile([K, K + 2], F32)
            nc.vector.tensor_copy(out=ss_sb[:], in_=ps_ss[:])
            nc.sync.dma_start(out=ss_out[:, :], in_=ss_sb[:])
            # vol: partition-reduce via matmul with ones
            va_r = cp.tile([P, 1], F32R)
            nc.vector.tensor_copy(out=va_r[:], in_=vol_acc[:])
            onesp = cp.tile([P, 2], F32R)
            nc.vector.tensor_copy(out=onesp[:, :1], in_=ones_col[:])
            nc.vector.tensor_copy(out=onesp[:, 1:], in_=ones_col[:])
            ps_v = pp.tile([1, 2], F32, tag="v")
            nc.tensor.matmul(out=ps_v[:], lhsT=va_r[:], rhs=onesp[:],
                             start=True, stop=True)
            v_sb = sp.tile([1, 1], F32)
            nc.vector.tensor_copy(out=v_sb[:], in_=ps_v[:, :1])
            nc.sync.dma_start(out=vol_out[:, :], in_=v_sb[:])
    return nc


# ------------------------------------------------------------------ P2 build

def build_p2(class_caps):
    """class_caps: list of 16 ints (multiples of 128), same for all cores."""
    tot = sum(class_caps)
    nc = bacc.Bacc(None, num_devices=NCORE, num_swdge_queues=4)
    tbl_t = nc.dram_tensor("tbl", [NLINES, 128], BF16, kind="ExternalInput")
    ir_t = nc.dram_tensor("idx_r", [P, tot // 16], I16, kind="ExternalInput")
    ic_t = nc.dram_tensor("idx_c", [P, tot // 16], I16, kind="ExternalInput")
    adj_out = nc.dram_tensor("adj_out", [K, K], F32, kind="ExternalOutput")

    with tile.TileContext(nc) as tc:
        with (tc.tile_pool(name="sbuf", bufs=2) as sp,
              tc.tile_pool(name="idx", bufs=2) as ip,
              tc.tile_pool(name="psum", bufs=1, space="PSUM") as pp):
            ps_adj = pp.tile([K, K], F32, tag="adj")
            first = True
            chunks = []
            off = 0
            for k, cap in enumerate(class_caps):
                o = 0
                while o < cap:
                    n = min(CHUNK, cap - o)
                    chunks.append((k, off + o, n))
                    o += n
                off += cap
            last_i = len(chunks) - 1
            for ci, (k, start, n) in enumerate(chunks):
                q = ci % 4
                kr, kc = (k >> 2) & 3, k & 3
                itr = ip.tile([P, n // 16], I16, tag=f"ir{q}")
                nc.sync.dma_start(out=itr[:], in_=ir_t[:, start // 16:(start + n) // 16])
                itc = ip.tile([P, n // 16], I16, tag=f"ic{q}")
                nc.sync.dma_start(out=itc[:], in_=ic_t[:, start // 16:(start + n) // 16])
                gr = sp.tile([P, n // P, 32], BF16, tag=f"gr{q}")
                dma_gather_raw(nc.gpsimd, gr[:], tbl_t[:, 32 * kr:32 * kr + 32],
                               itr[:], n, 32, 128, queue_num=q)
                gc = sp.tile([P, n // P, 32], BF16, tag=f"gc{q}")
                dma_gather_raw(nc.gpsimd, gc[:], tbl_t[:, 32 * kc:32 * kc + 32],
                               itc[:], n, 32, 128, queue_num=q)
                for g in range(n // P):
                    nc.tensor.matmul(out=ps_adj[:], lhsT=gr[:, g, :K],
                                     rhs=gc[:, g, :K], start=first,
                                     stop=(ci == last_i and g == n // P - 1))
                    first = False
            adj_sb = sp.tile([K, K], F32, tag="adjsb")
            nc.vector.tensor_copy(out=adj_sb[:], in_=ps_adj[:])
            nc.sync.dma_start(out=adj_out[:, :], in_=adj_sb[:])
    return nc


# ------------------------------------------------------------------ P3 build

NPT = 1024                 # padded pair-rows (i*32+j, j padded to 32) -> 8 tiles


def build_p3(b2_1v, b2_2v):
    nc = bacc.Bacc(None, num_devices=NCORE)
    t_in = {}
    for nm, shp in (("zxp", [NCORE * K, D]), ("ssp", [NCORE * K, K + 2]),
                    ("adjp", [NCORE * K, K]), ("volp", [NCORE, 1]),
                    ("wp", [D, D]), ("bp", [1, D]),
                    ("w1a_1", [D, D]), ("w1b_1", [D, D]), ("b1_1", [1, D]),
                    ("w2_1", [1, D]), ("lw_1", [D, D]), ("lb_1", [1, D]),
                    ("w1a_2", [D, D]), ("w1b_2", [D, D]), ("b1_2", [1, D]),
                    ("w2_2", [1, D]), ("lw_2", [D, D]), ("lb_2", [1, D]),
                    ("wout", [D, P]), ("bout", [1, P]),
                    ("p1t", [K, NPT]), ("p2t", [K, NPT]),
                    ("g_m", [NPT, K]), ("gt_m", [K, NPT]), ("p2_m", [NPT, K]),
                    ("eye", [K, K])):
        t_in[nm] = nc.dram_tensor(nm, shp, F32, kind="ExternalInput")
    out_t = nc.dram_tensor("out", [K, P], F32, kind="ExternalOutput")
    mc_t = nc.dram_tensor("mincut", [1, 1], F32, kind="ExternalOutput")
    or_t = nc.dram_tensor("ortho", [1, 1], F32, kind="ExternalOutput")
    z_t = nc.dram_tensor("z", [K, D], F32, kind="ExternalOutput")

    with nc.allow_low_precision(reason="float32r operands"), \
         tile.TileContext(nc) as tc:
        with (tc.tile_pool(name="consts", bufs=1) as cp,
              tc.tile_pool(name="sbuf", bufs=1) as sp,
              tc.tile_pool(name="psum", bufs=1, space="PSUM") as pp,
              tc.tile_pool(name="psacc", bufs=1, space="PSUM") as pa):
            ident0 = cp.tile([P, P], F32)
            make_identity(nc, ident0[:])
            ident = cp.tile([P, P], F32R)
            nc.vector.tensor_copy(out=ident[:], in_=ident0[:])

            def load_c(nm, shape, dt=F32R, tag=None):
                t0 = sp.tile(shape, F32, tag="stg")
                src = t_in[nm]
                if len(shape) == 3:
                    nc.sync.dma_start(out=t0[:], in_=src.rearrange(
                        "(c p) n -> p c n", p=shape[0]))
                else:
                    nc.sync.dma_start(out=t0[:], in_=src[:, :])
                t1 = cp.tile(shape, dt, tag=(tag or nm))
                nc.vector.tensor_copy(out=t1[:], in_=t0[:])
                return t1

            def reduce_parts(nm, cols, tag):
                t0 = sp.tile([K, NCORE, cols], F32, tag=tag + "0")
                nc.sync.dma_start(out=t0[:], in_=t_in[nm].rearrange(
                    "(c k) n -> k c n", k=K))
                acc = cp.tile([K, cols], F32, tag=tag)
                nc.vector.tensor_copy(out=acc[:], in_=t0[:, 0, :])
                for c in range(1, NCORE):
                    nc.vector.tensor_add(out=acc[:], in0=acc[:], in1=t0[:, c, :])
                return acc

            zx = reduce_parts("zxp", D, "zx")           # [K, D]
            ssa = reduce_parts("ssp", K + 2, "ssa")     # [K, K+1]
            adj = reduce_parts("adjp", K, "adj")        # [K, K]
            volp0 = sp.tile([1, NCORE], F32, tag="volp0")
            nc.sync.dma_start(out=volp0[:], in_=t_in["volp"].rearrange(
                "(c o) n -> o (c n)", o=1))
            vol = cp.tile([1, 1], F32, tag="vol")
            nc.vector.reduce_sum(out=vol[:], in_=volp0[:], axis=AX)

            ones_f = cp.tile([P, 1], F32)
            nc.vector.memset(ones_f[:], 1.0)
            ones_fr = cp.tile([1, P], F32)
            nc.vector.memset(ones_fr[:], 1.0)
            ones_k = cp.tile([K, 2], F32R)
            nc.vector.tensor_copy(out=ones_k[:, :1], in_=ones_f[:K, :])
            nc.vector.tensor_copy(out=ones_k[:, 1:], in_=ones_f[:K, :])
            ones_row = cp.tile([1, K], F32R)
            nc.vector.tensor_copy(out=ones_row[:], in_=ones_fr[:, :K])
            ones_p1 = cp.tile([1, P], F32R)
            nc.vector.tensor_copy(out=ones_p1[:], in_=ones_fr[:])

            def transpose_k(src_r, tag):
                dst = cp.tile([P, 4, K], F32R, tag=tag)
                for c in range(4):
                    tp = pp.tile([P, K], F32R, tag="tpk")
                    nc.tensor.transpose(out=tp[:], in_=src_r[:, c * P:(c + 1) * P],
                                        identity=ident[:K, :K])
                    nc.vector.tensor_copy(out=dst[:, c, :], in_=tp[:])
                return dst

            def mm_kd(xT, w4, tag, width=D, bias=None):
                ps = pp.tile([K, width], F32, tag="mmps")
                for c in range(4):
                    nc.tensor.matmul(out=ps[:], lhsT=xT[:, c, :],
                                     rhs=w4[:, c, :width],
                                     start=(c == 0),
                                     stop=(c == 3 and bias is None))
                if bias is not None:
                    blhs, brhs = bias
                    nc.tensor.matmul(out=ps[:], lhsT=blhs[:], rhs=brhs[:, :width],
                                     start=False, stop=True)
                return ps

            wp4 = load_c("wp", [P, 4, D])
            bp_r = load_c("bp", [1, D])
            zx_r = cp.tile([K, D], F32R, tag="zxr")
            nc.vector.tensor_copy(out=zx_r[:], in_=zx[:])
            zxT = transpose_k(zx_r, "zxT")
            colsum = cp.tile([K, 2], F32R, tag="colsum")
            nc.vector.tensor_copy(out=colsum[:, :1], in_=ssa[:, K:K + 1])
            nc.vector.tensor_copy(out=colsum[:, 1:], in_=ssa[:, K:K + 1])
            ps_ct = pp.tile([2, K], F32R, tag="selps")
            nc.tensor.transpose(out=ps_ct[:], in_=colsum[:], identity=ident[:K, :K])
            colsumT = cp.tile([1, K], F32R, tag="colsumT")
            nc.vector.tensor_copy(out=colsumT[:], in_=ps_ct[:1, :])
            ps_z = mm_kd(zxT, wp4, "zmm", bias=(colsumT, bp_r))
            z_cur = cp.tile([K, D], F32R, tag="zcur")
            nc.vector.tensor_copy(out=z_cur[:], in_=ps_z[:])
            z_sb = sp.tile([K, D], F32, tag="zsb")
            nc.vector.tensor_copy(out=z_sb[:], in_=z_cur[:])
            nc.sync.dma_start(out=z_t[:, :], in_=z_sb[:])

            # losses
            eye_r = load_c("eye", [K, K], dt=F32)
            dif = sp.tile([K, K], F32, tag="dif")
            nc.vector.tensor_sub(out=dif[:], in0=ssa[:, :K], in1=eye_r[:])
            sq = sp.tile([K, K], F32, tag="sq")
            nc.vector.tensor_mul(out=sq[:], in0=dif[:], in1=dif[:])
            rs = sp.tile([K, 1], F32R, tag="rs3")
            nc.vector.reduce_sum(out=rs[:], in_=sq[:], axis=AX)
            ps_o = pp.tile([1, 2], F32, tag="tiny1")
            nc.tensor.matmul(out=ps_o[:], lhsT=rs[:], rhs=ones_k[:],
                             start=True, stop=True)
            orto = sp.tile([1, 1], F32, tag="orto")
            nc.scalar.activation(out=orto[:], in_=ps_o[:, :1],
                                 func=mybir.ActivationFunctionType.Sqrt)
            nc.sync.dma_start(out=or_t[:, :], in_=orto[:])

            diag = sp.tile([K, K], F32, tag="diag")
            nc.vector.tensor_mul(out=diag[:], in0=adj[:], in1=eye_r[:])
            trs = sp.tile([K, 1], F32R, tag="trs")
            nc.vector.reduce_sum(out=trs[:], in_=diag[:], axis=AX)
            ps_c = pp.tile([1, 2], F32, tag="tiny1")
            nc.tensor.matmul(out=ps_c[:], lhsT=trs[:], rhs=ones_k[:],
                             start=True, stop=True)
            den = sp.tile([1, 1], F32, tag="den")
            nc.vector.tensor_scalar(out=den[:], in0=vol[:], scalar1=EPS,
                                    scalar2=None, op0=mybir.AluOpType.add)
            rden = sp.tile([1, 1], F32, tag="rden")
            nc.vector.reciprocal(out=rden[:], in_=den[:])
            mcv = sp.tile([1, 1], F32, tag="mcv")
            nc.vector.tensor_mul(out=mcv[:], in0=ps_c[:, :1], in1=rden[:])
            nc.vector.tensor_scalar(out=mcv[:], in0=mcv[:], scalar1=-1.0,
                                    scalar2=None, op0=mybir.AluOpType.mult)
            nc.sync.dma_start(out=mc_t[:, :], in_=mcv[:])

            # mask / alphas
            mask = cp.tile([K, K], F32, tag="mask")
            nc.vector.tensor_scalar(out=mask[:], in0=adj[:], scalar1=0.0,
                                    scalar2=None, op0=mybir.AluOpType.is_gt)
            hnr = sp.tile([K, 1], F32, tag="hnr")
            nc.vector.reduce_sum(out=hnr[:], in_=mask[:], axis=AX)
            hn = sp.tile([K, 1], F32, tag="hn")
            nc.vector.tensor_scalar(out=hn[:], in0=hnr[:], scalar1=0.0,
                                    scalar2=None, op0=mybir.AluOpType.is_gt)
            a_agg = cp.tile([K, 1], F32, tag="a_agg")
            nc.vector.tensor_scalar(out=a_agg[:], in0=hn[:], scalar1=0.5,
                                    scalar2=None, op0=mybir.AluOpType.mult)
            a_x = cp.tile([K, 1], F32, tag="a_x")
            nc.vector.tensor_scalar(out=a_x[:], in0=a_agg[:], scalar1=-1.0,
                                    scalar2=1.0, op0=mybir.AluOpType.mult,
                                    op1=mybir.AluOpType.add)

            p1t = load_c("p1t", [K, NPT])
            p2t = load_c("p2t", [K, NPT])
            g_m0 = sp.tile([P, 8, K], F32, tag="stg")
            nc.sync.dma_start(out=g_m0[:], in_=t_in["g_m"].rearrange(
                "(t p) k -> p t k", p=P))
            g_m = cp.tile([P, 8, K], F32R, tag="gm")
            nc.vector.tensor_copy(out=g_m[:], in_=g_m0[:])
            p2_m0 = sp.tile([P, 8, K], F32, tag="stg")
            nc.sync.dma_start(out=p2_m0[:], in_=t_in["p2_m"].rearrange(
                "(t p) k -> p t k", p=P))
            p2_m = cp.tile([P, 8, K], F32, tag="p2m")
            nc.vector.tensor_copy(out=p2_m[:], in_=p2_m0[:])
            gt_m = load_c("gt_m", [K, NPT])
            adj_r = cp.tile([K, K], F32R, tag="adjr")
            nc.vector.tensor_copy(out=adj_r[:], in_=adj[:])

            def explainer(x_in, wa4, wb4, b1r, w2full, b2v, lw4, lbr, tag):
                xT = transpose_k(x_in, tag + "xT")
                ps_a = mm_kd(xT, wa4, tag + "amm", bias=(ones_row, b1r))
                a_r = cp.tile([K, D], F32R, tag="xar")
                nc.vector.tensor_copy(out=a_r[:], in_=ps_a[:])
                ps_b = mm_kd(xT, wb4, tag + "bmm")
                b_r = cp.tile([K, D], F32R, tag="xbr")
                nc.vector.tensor_copy(out=b_r[:], in_=ps_b[:])

                ps_sums = pa.tile([K, 2], F32, tag="xsums")
                m_ts = []
                for t in range(8):
                    ps_h = pp.tile([P, D], F32, tag="mmps")
                    nc.tensor.matmul(out=ps_h[:], lhsT=p1t[:, t * P:(t + 1) * P],
                                     rhs=a_r[:], start=True, stop=False)
                    nc.tensor.matmul(out=ps_h[:], lhsT=p2t[:, t * P:(t + 1) * P],
                                     rhs=b_r[:], start=False, stop=True)
                    h_t = sp.tile([P, D], F32, tag="xht")
                    nc.scalar.activation(out=h_t[:], in_=ps_h[:],
                                         func=mybir.ActivationFunctionType.Relu)
                    hw = sp.tile([P, D], F32, tag="xhw")
                    nc.vector.tensor_mul(out=hw[:], in0=h_t[:], in1=w2full[:])
                    lgt = sp.tile([P, 1], F32, tag="xlgt")
                    nc.vector.reduce_sum(out=lgt[:], in_=hw[:], axis=AX)
                    nc.vector.tensor_scalar(out=lgt[:], in0=lgt[:],
                                            scalar1=float(b2v), scalar2=None,
                                            op0=mybir.AluOpType.add)
                    sg = sp.tile([P, 1], F32, tag="xsg")
                    nc.scalar.activation(out=sg[:], in_=lgt[:],
                                         func=mybir.ActivationFunctionType.Sigmoid)
                    ps_sel = pp.tile([P, K], F32, tag="selps")
                    nc.tensor.matmul(out=ps_sel[:], lhsT=p1t[:, t * P:(t + 1) * P],
                                     rhs=adj_r[:], start=True, stop=True)
                    selm = sp.tile([P, K], F32, tag="xselm")
                    nc.vector.tensor_mul(out=selm[:], in0=ps_sel[:],
                                         in1=p2_m[:, t, :])
                    adjf = sp.tile([P, 1], F32, tag="xadjf")
                    nc.vector.reduce_sum(out=adjf[:], in_=selm[:], axis=AX)
                    mkf = sp.tile([P, 1], F32, tag="xmkf")
                    nc.vector.tensor_scalar(out=mkf[:], in0=adjf[:], scalar1=0.0,
                                            scalar2=None, op0=mybir.AluOpType.is_gt)
                    m_t = cp.tile([P, 2], F32R, tag=f"xm{t}")
                    nc.vector.tensor_mul(out=m_t[:, :1], in0=sg[:], in1=mkf[:])
                    nc.vector.tensor_copy(out=m_t[:, 1:], in_=m_t[:, :1].bitcast(F32))
                    m_ts.append(m_t)
                    nc.tensor.matmul(out=ps_sums[:], lhsT=g_m[:, t, :],
                                     rhs=m_t[:], start=(t == 0), stop=(t == 7))
                inv = cp.tile([K, 2], F32R, tag="xinv")
                tmp = sp.tile([K, 1], F32, tag="xtmpi")
                nc.vector.tensor_scalar(out=tmp[:], in0=ps_sums[:, :1], scalar1=EPS,
                                        scalar2=None, op0=mybir.AluOpType.add)
                nc.vector.reciprocal(out=inv[:, :1], in_=tmp[:])
                nc.vector.tensor_copy(out=inv[:, 1:], in_=inv[:, :1].bitcast(F32))
                ps_agg = pa.tile([K, D], F32, tag="xagg")
                for t in range(8):
                    ps_ie = pp.tile([P, 2], F32, tag="selps")
                    nc.tensor.matmul(out=ps_ie[:], lhsT=gt_m[:, t * P:(t + 1) * P],
                                     rhs=inv[:], start=True, stop=True)
                    w_t = sp.tile([P, 1], F32, tag="xwt")
                    nc.vector.tensor_mul(out=w_t[:], in0=m_ts[t][:, :1].bitcast(F32),
                                         in1=ps_ie[:, :1])
                    ps_xe = pp.tile([P, D], F32, tag="mmps")
                    nc.tensor.matmul(out=ps_xe[:], lhsT=p2t[:, t * P:(t + 1) * P],
                                     rhs=x_in[:], start=True, stop=True)
                    wz = sp.tile([P, D], F32R, tag="xwz")
                    nc.vector.tensor_scalar_mul(out=wz[:], in0=ps_xe[:],
                                                scalar1=w_t[:])
                    nc.tensor.matmul(out=ps_agg[:], lhsT=g_m[:, t, :], rhs=wz[:],
                                     start=(t == 0), stop=(t == 7))
                emb = cp.tile([K, D], F32R, tag="xemb")
                t1 = sp.tile([K, D], F32, tag="xt1")
                nc.vector.tensor_scalar_mul(out=t1[:], in0=ps_agg[:],
                                            scalar1=a_agg[:])
                t2 = sp.tile([K, D], F32, tag="xt2")
                nc.vector.tensor_scalar_mul(out=t2[:], in0=x_in[:].bitcast(F32),
                                            scalar1=a_x[:])
                nc.vector.tensor_add(out=emb[:], in0=t1[:], in1=t2[:])
                embT = transpose_k(emb, tag + "embT")
                ps_hn = mm_kd(embT, lw4, tag + "hnmm", bias=(ones_row, lbr))
                h_next = cp.tile([K, D], F32R, tag=tag + "hnext")
                nc.vector.tensor_scalar_max(out=h_next[:], in0=ps_hn[:], scalar1=0.0)
                return h_next

            def w2_full(w2r, tag):
                ps_w = pp.tile([P, D], F32, tag="mmps")
                nc.tensor.matmul(out=ps_w[:], lhsT=ones_p1[:], rhs=w2r[:],
                                 start=True, stop=True)
                wf = cp.tile([P, D], F32, tag=tag)
                nc.vector.tensor_copy(out=wf[:], in_=ps_w[:])
                return wf

            w1a1 = load_c("w1a_1", [P, 4, D]); w1b1 = load_c("w1b_1", [P, 4, D])
            b11 = load_c("b1_1", [1, D]); w21 = load_c("w2_1", [1, D])
            lw1 = load_c("lw_1", [P, 4, D]); lb1 = load_c("lb_1", [1, D])
            w1a2 = load_c("w1a_2", [P, 4, D]); w1b2 = load_c("w1b_2", [P, 4, D])
            b12 = load_c("b1_2", [1, D]); w22 = load_c("w2_2", [1, D])
            lw2 = load_c("lw_2", [P, 4, D]); lb2 = load_c("lb_2", [1, D])
            wo4 = load_c("wout", [P, 4, P]); bo_r = load_c("bout", [1, P])
            w2f1 = w2_full(w21, "w2f1")
            w2f2 = w2_full(w22, "w2f2")

            h1 = explainer(z_cur, w1a1, w1b1, b11, w2f1, b2_1v, lw1, lb1, "e1")
            h2 = explainer(h1, w1a2, w1b2, b12, w2f2, b2_2v, lw2, lb2, "e2")
            h2T = transpose_k(h2, "h2T")
            ps_out = mm_kd(h2T, wo4, "omm", width=P, bias=(ones_row, bo_r))
            out_sb = sp.tile([K, P], F32, tag="outsb")
            nc.vector.tensor_copy(out=out_sb[:], in_=ps_out[:])
            nc.sync.dma_start(out=out_t[:, :], in_=out_sb[:])
    return nc


# ------------------------------------------------------------- host pipeline

_cache = {}


def kernel(x, edge_index, Wa, ba, Wp, bp,
           e1_W1, e1_b1, e1_W2, e1_b2, e1_lW, e1_lb,
           e2_W1, e2_b1, e2_W2, e2_b2, e2_lW, e2_lb,
           Wout, bout):
    x = np.asarray(x, np.float32)
    edge_index = np.asarray(edge_index)
    row = edge_index[0].astype(np.int64)
    col = edge_index[1].astype(np.int64)

    owner_r = row // SH_REAL
    owner_c = col // SH_REAL
    pos_r = owner_r * SH + (row - owner_r * SH_REAL)
    pos_c = owner_c * SH + (col - owner_c * SH_REAL)
    deg = np.bincount(row, minlength=N).astype(np.float32)

    xp = np.zeros((NCORE, SH, D), np.float32)
    degp = np.zeros((NCORE, SH, 1), np.float32)
    for m in range(NCORE):
        xp[m, :SH_REAL] = x[m * SH_REAL:(m + 1) * SH_REAL]
        degp[m, :SH_REAL, 0] = deg[m * SH_REAL:(m + 1) * SH_REAL]
    wa_in = np.asarray(Wa, np.float32)
    ba_in = np.asarray(ba, np.float32).reshape(1, K)
    if "p1" not in _cache:
        _cache["p1"] = SpmdRunner(build_p1())
    res1 = _cache["p1"]([dict(x=xp[m], wa=wa_in, ba=ba_in, deg=degp[m])
                         for m in range(NCORE)])

    s_full = np.concatenate([res1[m]["s_out"].reshape(SH, K)[:SH_REAL]
                             for m in range(NCORE)], axis=0)
    table = np.concatenate([res1[m]["sb_out"].reshape(SH, 32)
                            for m in range(NCORE)], axis=0)
    table_lines = np.zeros((NLINES, 128), ml_dtypes.bfloat16)
    table_lines[:NPOS // 4] = table.reshape(NPOS // 4, 128)

    cls = ((pos_r & 3) * 4 + (pos_c & 3)).astype(np.int64)
    key = owner_r * 16 + cls
    order = np.argsort(key, kind="stable")
    key_s = key[order]
    pr_s = (pos_r[order] >> 2).astype(np.int16)
    pc_s = (pos_c[order] >> 2).astype(np.int16)
    counts = np.bincount(key_s, minlength=NCORE * 16).reshape(NCORE, 16)
    caps = ((counts.max(axis=0) + 127) // 128) * 128
    caps = np.maximum(caps, 128)
    tot = int(caps.sum())
    pad_idx = np.int16(NPOS // 4)
    ir = np.full((NCORE, tot), pad_idx, np.int16)
    ic = np.full((NCORE, tot), pad_idx, np.int16)
    starts = np.concatenate([[0], np.cumsum(counts.reshape(-1))])[:-1].reshape(NCORE, 16)
    offs = np.concatenate([[0], np.cumsum(caps)])[:-1].astype(np.int64)
    for m in range(NCORE):
        for k in range(16):
            cnt = counts[m, k]
            s0 = starts[m, k]
            ir[m, offs[k]:offs[k] + cnt] = pr_s[s0:s0 + cnt]
            ic[m, offs[k]:offs[k] + cnt] = pc_s[s0:s0 + cnt]
    ir_w = np.stack([wrap_idxs(ir[m]) for m in range(NCORE)])
    ic_w = np.stack([wrap_idxs(ic[m]) for m in range(NCORE)])

    caps_t = tuple(int(c) for c in caps)
    ck = ("p2", caps_t)
    if ck not in _cache:
        _cache[ck] = SpmdRunner(build_p2(list(caps_t)))
    res2 = _cache[ck]([dict(tbl=table_lines, idx_r=ir_w[m], idx_c=ic_w[m])
                       for m in range(NCORE)])

    i_of = np.arange(NPT) // 32
    j_of = np.arange(NPT) % 32
    valid = (i_of < K) & (j_of < K)
    p1t = np.zeros((K, NPT), np.float32)
    p1t[np.minimum(i_of, K - 1), np.arange(NPT)] = valid.astype(np.float32)
    p2t = np.zeros((K, NPT), np.float32)
    p2t[np.minimum(j_of, K - 1), np.arange(NPT)] = valid.astype(np.float32)
    g_m = np.zeros((NPT, K), np.float32)
    g_m[np.arange(NPT), np.minimum(i_of, K - 1)] = valid.astype(np.float32)
    gt_m = np.ascontiguousarray(g_m.T)
    p2_m = np.zeros((NPT, K), np.float32)
    p2_m[np.arange(NPT), np.minimum(j_of, K - 1)] = valid.astype(np.float32)

    f32 = lambda a: np.ascontiguousarray(np.asarray(a, np.float32))
    in3 = dict(
        zxp=np.concatenate([res1[m]["zx_out"] for m in range(NCORE)], 0),
        ssp=np.concatenate([res1[m]["ss_out"] for m in range(NCORE)], 0),
        adjp=np.concatenate([res2[m]["adj_out"] for m in range(NCORE)], 0),
        volp=np.concatenate([res1[m]["vol_out"] for m in range(NCORE)], 0),
        wp=f32(Wp), bp=f32(bp).reshape(1, D),
        w1a_1=f32(np.asarray(e1_W1)[:D]), w1b_1=f32(np.asarray(e1_W1)[D:]),
        b1_1=f32(e1_b1).reshape(1, D), w2_1=f32(e1_W2).reshape(1, D),
        lw_1=f32(e1_lW), lb_1=f32(e1_lb).reshape(1, D),
        w1a_2=f32(np.asarray(e2_W1)[:D]), w1b_2=f32(np.asarray(e2_W1)[D:]),
        b1_2=f32(e2_b1).reshape(1, D), w2_2=f32(e2_W2).reshape(1, D),
        lw_2=f32(e2_lW), lb_2=f32(e2_lb).reshape(1, D),
        wout=f32(Wout), bout=f32(bout).reshape(1, P),
        p1t=p1t, p2t=p2t, g_m=g_m, gt_m=gt_m, p2_m=p2_m,
        eye=np.eye(K, dtype=np.float32),
    )
    b2_1v = float(np.asarray(e1_b2).reshape(-1)[0])
    b2_2v = float(np.asarray(e2_b2).reshape(-1)[0])
    pk = ("p3", round(b2_1v, 9), round(b2_2v, 9))
    if pk not in _cache:
        _cache[pk] = SpmdRunner(build_p3(b2_1v, b2_2v))
    res3 = _cache[pk]([in3] * NCORE)
    out = res3[0]["out"]
    mincut = np.float32(res3[0]["mincut"][0, 0])
    ortho = np.float32(res3[0]["ortho"][0, 0])
    Z = res3[0]["z"]
    return (out, mincut, ortho, Z, s_full)


# revision 15
# speedup vs baseline: 25.0195x; 25.0195x over previous
"""Trainium2 Bass kernel for nn_MinCutExplainerGNN (8 NeuronCores, SPMD).

Structure (3 SPMD launches, no collectives):
  P1 (x-phase, nodes sharded 8-way): S = softmax(x@Wa+ba); partials
     ZX = S^T x, SSaug = S^T [S|1], vol = sum(deg * rowsum(S)); S fp32 out +
     bf16 gather table shard.
  host: concatenates the bf16 S table (pure data movement), edge index
     preprocessing (integer only: bucketing by destination shard, class split
     by (pos_r&3, pos_c&3) for 256B-line gather addressing, padding).
  P2 (edge phase, edges sharded by destination): two-sided dma_gather of S
     rows (64B bf16 payloads out of 256B-stride lines), adj partial
     accumulation via PSUM outer-product matmuls.
  P3 (replicated finisher): reduce partials, Z = ZX@Wp + colsum*bp, losses,
     the two dense-K explainer layers, final linear.
"""
import numpy as np
import ml_dtypes
import jax
from jax.sharding import Mesh, PartitionSpec
from jax.experimental.shard_map import shard_map

import concourse.bacc as bacc
import concourse.bass as bass
import concourse.mybir as mybir
import concourse.tile as tile
from concourse import ap_utils
from concourse.bass import MemorySpace
from concourse._compat import exact_div
from concourse import bass2jax
from concourse.bass2jax import _bass_exec_p, partition_id_tensor
from concourse.masks import make_identity

P = 128
NCORE = 8
N = 100000
D = 512
K = 30
EPS = 1e-9
SH_REAL = 12500            # real rows per shard
SH = 12544                 # padded rows per shard (98 * 128)
TILES = SH // P            # 98
NPOS = NCORE * SH          # 100352
NLINES = NPOS // 4 + 128   # table lines (4 rows/256B line) + zero pad lines
F32, F32R, BF16, I16 = (mybir.dt.float32, mybir.dt.float32r,
                        mybir.dt.bfloat16, mybir.dt.int16)
AX = mybir.AxisListType.X
CHUNK = 16128              # idxs per dma_gather instruction (scratch cap ~16200)


# ---------------------------------------------------------------- utilities

def dma_gather_raw(gp, out_ap, in_ap, idxs_ap, num_idxs, elem_size, elem_step,
                   queue_num=0):
    gp._assert_queue_num(queue_num)
    assert idxs_ap.dtype == I16
    assert in_ap.space == MemorySpace.DRAM
    assert in_ap.dtype == out_ap.dtype
    assert num_idxs % 128 == 0
    assert ap_utils.ap_is_contiguous(out_ap.ap[1:])
    assert ap_utils.ap_is_contiguous(idxs_ap.ap[1:])
    assert in_ap.ap[-1][1] == out_ap.ap[-1][1] == elem_size
    assert out_ap.ap[0][1] * out_ap.ap[1][1] == num_idxs
    assert in_ap.ap[0][0] == elem_step
    stride_bytes_256 = exact_div(elem_step * mybir.dt.size(in_ap.dtype), 256)
    _in_ap = gp.lower_ap_dma(in_ap, for_custom_bir_dma=True)
    return gp.add_instruction(
        mybir.InstDMAGatherAnt(
            name=gp.bass.get_next_instruction_name(),
            ins=[*_in_ap, gp.lower_ap(idxs_ap),
                 gp.lower_val_access(gp.to_reg(num_idxs))],
            outs=[gp.lower_ap(out_ap)],
            transpose=False, num_idxs=num_idxs, elem_size=elem_size,
            stride_bytes_256=stride_bytes_256, gen_mode=0, single_packet=False,
            queue_num=queue_num, sbuf_tokens_per_rank=0,
            sbuf_free_dim_per_rank=0, sbuf_free_dim_pad_per_rank=0,
            sbuf_byte_offset=0,
        ))


def wrap_idxs(idx):
    n = idx.shape[0]
    w = idx.astype(np.int16).reshape(n // 16, 16).T
    return np.tile(w, (8, 1))


class SpmdRunner:
    def __init__(self, nc, n_cores=NCORE):
        if not nc.is_finalized():
            nc.finalize()
        self.nc, self.n_cores = nc, n_cores
        bass2jax.install_neuronx_cc_hook()
        pname = nc.partition_id_tensor.name if nc.partition_id_tensor else None
        in_names, out_names, out_avals, zouts = [], [], [], []
        for alloc in nc.m.functions[0].allocations:
            if not isinstance(alloc, mybir.MemoryLocationSet):
                continue
            name = alloc.memorylocations[0].name
            if alloc.kind == "ExternalInput":
                if name != pname:
                    in_names.append(name)
            elif alloc.kind == "ExternalOutput":
                shape, dtype = tuple(alloc.tensor_shape), mybir.dt.np(alloc.dtype)
                out_names.append(name)
                out_avals.append(jax.core.ShapedArray(shape, dtype))
                zouts.append((shape, dtype))
        self.in_names, self.out_names = in_names, out_names
        self.out_avals, self.zero_outs = out_avals, zouts
        n_params, n_outs = len(in_names), len(out_avals)
        all_in = list(in_names) + list(out_names) + ([pname] if pname else [])

        def _body(*args):
            operands = list(args)
            if pname is not None:
                operands.append(partition_id_tensor())
            return tuple(_bass_exec_p.bind(
                *operands, out_avals=tuple(out_avals), in_names=tuple(all_in),
                out_names=tuple(out_names), lowering_input_output_aliases=(),
                sim_require_finite=True, sim_require_nnan=True, nc=nc))

        devices = jax.devices()[:n_cores]
        mesh = Mesh(np.asarray(devices), ("core",))
        self.sharded = jax.jit(
            shard_map(_body, mesh=mesh,
                      in_specs=(PartitionSpec("core"),) * (n_params + n_outs),
                      out_specs=(PartitionSpec("core"),) * n_outs,
                      check_rep=False),
            donate_argnums=tuple(range(n_params, n_params + n_outs)),
            keep_unused=True)

    def __call__(self, in_maps):
        concat_in = [np.concatenate([np.asarray(in_maps[c][k])
                                     for c in range(self.n_cores)], axis=0)
                     for k in self.in_names]
        zeros = [np.zeros((self.n_cores * s[0], *s[1:]), d)
                 for (s, d) in self.zero_outs]
        outs = self.sharded(*concat_in, *zeros)
        jax.block_until_ready(outs)
        return [{k: np.asarray(outs[i]).reshape(self.n_cores,
                                                *self.out_avals[i].shape)[c]
                 for i, k in enumerate(self.out_names)}
                for c in range(self.n_cores)]


# ------------------------------------------------------------------ P1 build

def build_p1():
    nc = bacc.Bacc(None, num_devices=NCORE)
    x_t = nc.dram_tensor("x", [SH, D], F32, kind="ExternalInput")
    wa_t = nc.dram_tensor("wa", [D, K], F32, kind="ExternalInput")
    ba_t = nc.dram_tensor("ba", [1, K], F32, kind="ExternalInput")
    deg_t = nc.dram_tensor("deg", [SH, 1], F32, kind="ExternalInput")
    s_out = nc.dram_tensor("s_out", [TILES, P, K], F32, kind="ExternalOutput")
    sb_out = nc.dram_tensor("sb_out", [TILES, P, 32], BF16, kind="ExternalOutput")
    zx_out = nc.dram_tensor("zx_out", [K, D], F32, kind="ExternalOutput")
    ss_out = nc.dram_tensor("ss_out", [K, K + 2], F32, kind="ExternalOutput")
    vol_out = nc.dram_tensor("vol_out", [1, 1], F32, kind="ExternalOutput")

    with nc.allow_low_precision(reason="float32r operands"), \
         tile.TileContext(nc) as tc:
        with (tc.tile_pool(name="consts", bufs=1) as cp,
              tc.tile_pool(name="sbuf", bufs=3) as sp,
              tc.tile_pool(name="psum", bufs=2, space="PSUM") as pp,
              tc.tile_pool(name="psacc", bufs=1, space="PSUM") as pa):
            ident0 = cp.tile([P, P], F32)
            make_identity(nc, ident0[:])
            ident = cp.tile([P, P], F32R)
            nc.vector.tensor_copy(out=ident[:], in_=ident0[:])
            wa0 = cp.tile([P, 4, K], F32)
            nc.sync.dma_start(out=wa0[:], in_=wa_t.rearrange("(c p) k -> p c k", p=P))
            wa = cp.tile([P, 4, K], F32R)
            nc.vector.tensor_copy(out=wa[:], in_=wa0[:])
            ones1f = cp.tile([1, P], F32)
            nc.vector.memset(ones1f[:], 1.0)
            ones1 = cp.tile([1, P], F32R)
            nc.vector.tensor_copy(out=ones1[:], in_=ones1f[:])
            ba0 = cp.tile([1, K], F32)
            nc.sync.dma_start(out=ba0[:], in_=ba_t[:, :])
            ba = cp.tile([1, K], F32R)
            nc.vector.tensor_copy(out=ba[:], in_=ba0[:])
            vol_acc = cp.tile([P, 1], F32)
            nc.vector.memset(vol_acc[:], 0.0)
            ones_col = cp.tile([P, 1], F32)
            nc.vector.memset(ones_col[:], 1.0)
            zero_col = cp.tile([P, 1], F32)
            nc.vector.memset(zero_col[:], 0.0)
            valid = cp.tile([P, 1], F32)
            nc.vector.memset(valid[:], 1.0)
            nc.gpsimd.affine_select(out=valid[:], in_=valid[:],
                                    compare_op=mybir.AluOpType.is_ge,
                                    fill=0.0, base=SH_REAL - 97 * P - 1,
                                    pattern=[[0, 1]], channel_multiplier=-1)
            ps_zx = pa.tile([K, D], F32, tag="zx")
            ps_ss = pa.tile([K, K + 2], F32, tag="ss")

            for t in range(TILES):
                xt0 = sp.tile([P, D], F32, tag="x0")
                nc.sync.dma_start(out=xt0[:], in_=x_t[t * P:(t + 1) * P, :])
                xt = sp.tile([P, D], F32R, tag="x")
                nc.vector.tensor_copy(out=xt[:], in_=xt0[:])
                xT = sp.tile([P, 4, P], F32R, tag="xT")
                for c in range(4):
                    tp = pp.tile([P, P], F32R, tag="tp")
                    nc.tensor.transpose(out=tp[:], in_=xt[:, c * P:(c + 1) * P],
                                        identity=ident[:])
                    nc.vector.tensor_copy(out=xT[:, c, :], in_=tp[:])
                # logits [P, K] = x @ Wa + ba
                lg = pp.tile([P, K], F32, tag="lg")
                for c in range(4):
                    nc.tensor.matmul(out=lg[:], lhsT=xT[:, c, :], rhs=wa[:, c, :],
                                     start=(c == 0), stop=False)
                nc.tensor.matmul(out=lg[:], lhsT=ones1[:], rhs=ba[:],
                                 start=False, stop=True)
                ex = sp.tile([P, K], F32, tag="ex")
                nc.scalar.activation(out=ex[:], in_=lg[:],
                                     func=mybir.ActivationFunctionType.Exp)
                ssum = sp.tile([P, 1], F32, tag="ssum")
                nc.vector.reduce_sum(out=ssum[:], in_=ex[:], axis=AX)
                rcp = sp.tile([P, 1], F32, tag="rcp")
                nc.vector.reciprocal(out=rcp[:], in_=ssum[:])
                s_f = sp.tile([P, K], F32R, tag="sf")
                nc.vector.tensor_scalar_mul(out=s_f[:], in0=ex[:], scalar1=rcp[:])
                if t == TILES - 1:
                    # zero padding rows 12500..12543 (partitions 84..127)
                    nc.vector.tensor_scalar_mul(out=s_f[:], in0=s_f[:].bitcast(F32),
                                                scalar1=valid[:])
                # outputs: S fp32 + bf16 table shard
                nc.sync.dma_start(out=s_out[t, :, :], in_=s_f[:].bitcast(F32))
                s_b = sp.tile([P, 32], BF16, tag="sb")
                nc.vector.memset(s_b[:, K:], 0.0)
                nc.vector.tensor_copy(out=s_b[:, :K], in_=s_f[:])
                nc.sync.dma_start(out=sb_out[t, :, :], in_=s_b[:])
                # ZX += S^T x ; SSaug += S^T [S|1]
                nc.tensor.matmul(out=ps_zx[:], lhsT=s_f[:], rhs=xt[:],
                                 start=(t == 0), stop=(t == TILES - 1))
                s_aug = sp.tile([P, K + 2], F32R, tag="saug")
                nc.vector.tensor_copy(out=s_aug[:, :K], in_=s_f[:])
                nc.vector.tensor_copy(out=s_aug[:, K + 1:], in_=zero_col[:])
                if t == TILES - 1:
                    nc.vector.tensor_copy(out=s_aug[:, K:K + 1], in_=valid[:])
                else:
                    nc.vector.tensor_copy(out=s_aug[:, K:K + 1], in_=ones_col[:])
                nc.tensor.matmul(out=ps_ss[:], lhsT=s_f[:], rhs=s_aug[:],
                                 start=(t == 0), stop=(t == TILES - 1))
                # vol partial: sum(deg * rowsum(S))
                dg = sp.tile([P, 1], F32, tag="dg")
                nc.sync.dma_start(out=dg[:], in_=deg_t[t * P:(t + 1) * P, :])
                rs = sp.tile([P, 1], F32, tag="rs")
                nc.vector.reduce_sum(out=rs[:], in_=s_f[:].bitcast(F32), axis=AX)
                dv = sp.tile([P, 1], F32, tag="dv")
                nc.vector.tensor_mul(out=dv[:], in0=rs[:], in1=dg[:])
                nc.vector.tensor_add(out=vol_acc[:], in0=vol_acc[:], in1=dv[:])

            zx_sb = sp.tile([K, D], F32)
            nc.vector.tensor_copy(out=zx_sb[:], in_=ps_zx[:])
            nc.sync.dma_start(out=zx_out[:, :], in_=zx_sb[:])
            ss_sb = sp.tile([K, K + 2], F32)
            nc.vector.tensor_copy(out=ss_sb[:], in_=ps_ss[:])
            nc.sync.dma_start(out=ss_out[:, :], in_=ss_sb[:])
            # vol: partition-reduce via matmul with ones
            va_r = cp.tile([P, 1], F32R)
            nc.vector.tensor_copy(out=va_r[:], in_=vol_acc[:])
            onesp = cp.tile([P, 2], F32R)
            nc.vector.tensor_copy(out=onesp[:, :1], in_=ones_col[:])
            nc.vector.tensor_copy(out=onesp[:, 1:], in_=ones_col[:])
            ps_v = pp.tile([1, 2], F32, tag="v")
            nc.tensor.matmul(out=ps_v[:], lhsT=va_r[:], rhs=onesp[:],
                             start=True, stop=True)
            v_sb = sp.tile([1, 1], F32)
            nc.vector.tensor_copy(out=v_sb[:], in_=ps_v[:, :1])
            nc.sync.dma_start(out=vol_out[:, :], in_=v_sb[:])
    return nc


# ------------------------------------------------------------------ P2 build

def build_p2(class_caps):
    """class_caps: list of 16 ints (multiples of 128), same for all cores."""
    tot = sum(class_caps)
    nc = bacc.Bacc(None, num_devices=NCORE, num_swdge_queues=4)
    tbl_t = nc.dram_tensor("tbl", [NLINES, 128], BF16, kind="ExternalInput")
    ir_t = nc.dram_tensor("idx_r", [P, tot // 16], I16, kind="ExternalInput")
    ic_t = nc.dram_tensor("idx_c", [P, tot // 16], I16, kind="ExternalInput")
    adj_out = nc.dram_tensor("adj_out", [K, K], F32, kind="ExternalOutput")

    with tile.TileContext(nc) as tc:
        with (tc.tile_pool(name="sbuf", bufs=2) as sp,
              tc.tile_pool(name="idx", bufs=2) as ip,
              tc.tile_pool(name="psum", bufs=1, space="PSUM") as pp):
            ps_adj = pp.tile([K, K], F32, tag="adj")
            first = True
            chunks = []
            off = 0
            for k, cap in enumerate(class_caps):
                o = 0
                while o < cap:
                    n = min(CHUNK, cap - o)
                    chunks.append((k, off + o, n))
                    o += n
                off += cap
            last_i = len(chunks) - 1
            for ci, (k, start, n) in enumerate(chunks):
                q = ci % 4
                kr, kc = (k >> 2) & 3, k & 3
                itr = ip.tile([P, n // 16], I16, tag=f"ir{q}")
                nc.sync.dma_start(out=itr[:], in_=ir_t[:, start // 16:(start + n) // 16])
                itc = ip.tile([P, n // 16], I16, tag=f"ic{q}")
                nc.sync.dma_start(out=itc[:], in_=ic_t[:, start // 16:(start + n) // 16])
                gr = sp.tile([P, n // P, 32], BF16, tag=f"gr{q}")
                dma_gather_raw(nc.gpsimd, gr[:], tbl_t[:, 32 * kr:32 * kr + 32],
                               itr[:], n, 32, 128, queue_num=q)
                gc = sp.tile([P, n // P, 32], BF16, tag=f"gc{q}")
                dma_gather_raw(nc.gpsimd, gc[:], tbl_t[:, 32 * kc:32 * kc + 32],
                               itc[:], n, 32, 128, queue_num=q)
                for g in range(n // P):
                    nc.tensor.matmul(out=ps_adj[:], lhsT=gr[:, g, :K],
                                     rhs=gc[:, g, :K], start=first,
                                     stop=(ci == last_i and g == n // P - 1))
                    first = False
            adj_sb = sp.tile([K, K], F32, tag="adjsb")
            nc.vector.tensor_copy(out=adj_sb[:], in_=ps_adj[:])
            nc.sync.dma_start(out=adj_out[:, :], in_=adj_sb[:])
    return nc


# ------------------------------------------------------------------ P3 build

NPT = 1024                 # padded pair-rows (i*32+j, j padded to 32) -> 8 tiles


def build_p3(b2_1v, b2_2v):
    nc = bacc.Bacc(None, num_devices=NCORE)
    t_in = {}
    for nm, shp in (("zxp", [NCORE * K, D]), ("ssp", [NCORE * K, K + 2]),
                    ("adjp", [NCORE * K, K]), ("volp", [NCORE, 1]),
                    ("wp", [D, D]), ("bp", [1, D]),
                    ("w1a_1", [D, D]), ("w1b_1", [D, D]), ("b1_1", [1, D]),
                    ("w2_1", [1, D]), ("lw_1", [D, D]), ("lb_1", [1, D]),
                    ("w1a_2", [D, D]), ("w1b_2", [D, D]), ("b1_2", [1, D]),
                    ("w2_2", [1, D]), ("lw_2", [D, D]), ("lb_2", [1, D]),
                    ("wout", [D, P]), ("bout", [1, P]),
                    ("p1t", [K, NPT]), ("p2t", [K, NPT]),
                    ("g_m", [NPT, K]), ("gt_m", [K, NPT]), ("p2_m", [NPT, K]),
                    ("eye", [K, K])):
        t_in[nm] = nc.dram_tensor(nm, shp, F32, kind="ExternalInput")
    out_t = nc.dram_tensor("out", [K, P], F32, kind="ExternalOutput")
    mc_t = nc.dram_tensor("mincut", [1, 1], F32, kind="ExternalOutput")
    or_t = nc.dram_tensor("ortho", [1, 1], F32, kind="ExternalOutput")
    z_t = nc.dram_tensor("z", [K, D], F32, kind="ExternalOutput")

    with nc.allow_low_precision(reason="float32r operands"), \
         tile.TileContext(nc) as tc:
        with (tc.tile_pool(name="consts", bufs=1) as cp,
              tc.tile_pool(name="sbuf", bufs=1) as sp,
              tc.tile_pool(name="psum", bufs=1, space="PSUM") as pp,
              tc.tile_pool(name="psacc", bufs=1, space="PSUM") as pa):
            ident0 = cp.tile([P, P], F32)
            make_identity(nc, ident0[:])
            ident = cp.tile([P, P], F32R)
            nc.vector.tensor_copy(out=ident[:], in_=ident0[:])

            def load_c(nm, shape, dt=F32R, tag=None):
                t0 = sp.tile(shape, F32, tag="stg")
                src = t_in[nm]
                if len(shape) == 3:
                    nc.sync.dma_start(out=t0[:], in_=src.rearrange(
                        "(c p) n -> p c n", p=shape[0]))
                else:
                    nc.sync.dma_start(out=t0[:], in_=src[:, :])
                t1 = cp.tile(shape, dt, tag=(tag or nm))
                nc.vector.tensor_copy(out=t1[:], in_=t0[:])
                return t1

            def reduce_parts(nm, cols, tag):
                t0 = sp.tile([K, NCORE, cols], F32, tag=tag + "0")
                nc.sync.dma_start(out=t0[:], in_=t_in[nm].rearrange(
                    "(c k) n -> k c n", k=K))
                acc = cp.tile([K, cols], F32, tag=tag)
                nc.vector.tensor_copy(out=acc[:], in_=t0[:, 0, :])
                for c in range(1, NCORE):
                    nc.vector.tensor_add(out=acc[:], in0=acc[:], in1=t0[:, c, :])
                return acc

            zx = reduce_parts("zxp", D, "zx")           # [K, D]
            ssa = reduce_parts("ssp", K + 2, "ssa")     # [K, K+1]
            adj = reduce_parts("adjp", K, "adj")        # [K, K]
            volp0 = sp.tile([1, NCORE], F32, tag="volp0")
            nc.sync.dma_start(out=volp0[:], in_=t_in["volp"].rearrange(
                "(c o) n -> o (c n)", o=1))
            vol = cp.tile([1, 1], F32, tag="vol")
            nc.vector.reduce_sum(out=vol[:], in_=volp0[:], axis=AX)

            ones_f = cp.tile([P, 1], F32)
            nc.vector.memset(ones_f[:], 1.0)
            ones_fr = cp.tile([1, P], F32)
            nc.vector.memset(ones_fr[:], 1.0)
            ones_k = cp.tile([K, 2], F32R)
            nc.vector.tensor_copy(out=ones_k[:, :1], in_=ones_f[:K, :])
            nc.vector.tensor_copy(out=ones_k[:, 1:], in_=ones_f[:K, :])
            ones_row = cp.tile([1, K], F32R)
            nc.vector.tensor_copy(out=ones_row[:], in_=ones_fr[:, :K])
            ones_p1 = cp.tile([1, P], F32R)
            nc.vector.tensor_copy(out=ones_p1[:], in_=ones_fr[:])

            def transpose_k(src_r, tag):
                dst = cp.tile([P, 4, K], F32R, tag=tag)
                for c in range(4):
                    tp = pp.tile([P, K], F32R, tag="tpk")
                    nc.tensor.transpose(out=tp[:], in_=src_r[:, c * P:(c + 1) * P],
                                        identity=ident[:K, :K])
                    nc.vector.tensor_copy(out=dst[:, c, :], in_=tp[:])
                return dst

            def mm_kd(xT, w4, tag, width=D, bias=None):
                ps = pp.tile([K, width], F32, tag="mmps")
                for c in range(4):
                    nc.tensor.matmul(out=ps[:], lhsT=xT[:, c, :],
                                     rhs=w4[:, c, :width],
                                     start=(c == 0),
                                     stop=(c == 3 and bias is None))
                if bias is not None:
                    blhs, brhs = bias
                    nc.tensor.matmul(out=ps[:], lhsT=blhs[:], rhs=brhs[:, :width],
                                     start=False, stop=True)
                return ps

            wp4 = load_c("wp", [P, 4, D])
            bp_r = load_c("bp", [1, D])
            zx_r = cp.tile([K, D], F32R, tag="zxr")
            nc.vector.tensor_copy(out=zx_r[:], in_=zx[:])
            zxT = transpose_k(zx_r, "zxT")
            colsum = cp.tile([K, 2], F32R, tag="colsum")
            nc.vector.tensor_copy(out=colsum[:, :1], in_=ssa[:, K:K + 1])
            nc.vector.tensor_copy(out=colsum[:, 1:], in_=ssa[:, K:K + 1])
            ps_ct = pp.tile([2, K], F32R, tag="selps")
            nc.tensor.transpose(out=ps_ct[:], in_=colsum[:], identity=ident[:K, :K])
            colsumT = cp.tile([1, K], F32R, tag="colsumT")
            nc.vector.tensor_copy(out=colsumT[:], in_=ps_ct[:1, :])
            ps_z = mm_kd(zxT, wp4, "zmm", bias=(colsumT, bp_r))
            z_cur = cp.tile([K, D], F32R, tag="zcur")
            nc.vector.tensor_copy(out=z_cur[:], in_=ps_z[:])
            z_sb = sp.tile([K, D], F32, tag="zsb")
            nc.vector.tensor_copy(out=z_sb[:], in_=z_cur[:])
            nc.sync.dma_start(out=z_t[:, :], in_=z_sb[:])

            # losses
            eye_r = load_c("eye", [K, K], dt=F32)
            dif = sp.tile([K, K], F32, tag="dif")
            nc.vector.tensor_sub(out=dif[:], in0=ssa[:, :K], in1=eye_r[:])
            sq = sp.tile([K, K], F32, tag="sq")
            nc.vector.tensor_mul(out=sq[:], in0=dif[:], in1=dif[:])
            rs = sp.tile([K, 1], F32R, tag="rs3")
            nc.vector.reduce_sum(out=rs[:], in_=sq[:], axis=AX)
            ps_o = pp.tile([1, 2], F32, tag="tiny1")
            nc.tensor.matmul(out=ps_o[:], lhsT=rs[:], rhs=ones_k[:],
                             start=True, stop=True)
            orto = sp.tile([1, 1], F32, tag="orto")
            nc.scalar.activation(out=orto[:], in_=ps_o[:, :1],
                                 func=mybir.ActivationFunctionType.Sqrt)
            nc.sync.dma_start(out=or_t[:, :], in_=orto[:])

            diag = sp.tile([K, K], F32, tag="diag")
            nc.vector.tensor_mul(out=diag[:], in0=adj[:], in1=eye_r[:])
            trs = sp.tile([K, 1], F32R, tag="trs")
            nc.vector.reduce_sum(out=trs[:], in_=diag[:], axis=AX)
            ps_c = pp.tile([1, 2], F32, tag="tiny1")
            nc.tensor.matmul(out=ps_c[:], lhsT=trs[:], rhs=ones_k[:],
                             start=True, stop=True)
            den = sp.tile([1, 1], F32, tag="den")
            nc.vector.tensor_scalar(out=den[:], in0=vol[:], scalar1=EPS,
                                    scalar2=None, op0=mybir.AluOpType.add)
            rden = sp.tile([1, 1], F32, tag="rden")
            nc.vector.reciprocal(out=rden[:], in_=den[:])
            mcv = sp.tile([1, 1], F32, tag="mcv")
            nc.vector.tensor_mul(out=mcv[:], in0=ps_c[:, :1], in1=rden[:])
            nc.vector.tensor_scalar(out=mcv[:], in0=mcv[:], scalar1=-1.0,
                                    scalar2=None, op0=mybir.AluOpType.mult)
            nc.sync.dma_start(out=mc_t[:, :], in_=mcv[:])

            # mask / alphas
            mask = cp.tile([K, K], F32, tag="mask")
            nc.vector.tensor_scalar(out=mask[:], in0=adj[:], scalar1=0.0,
                                    scalar2=None, op0=mybir.AluOpType.is_gt)
            hnr = sp.tile([K, 1], F32, tag="hnr")
            nc.vector.reduce_sum(out=hnr[:], in_=mask[:], axis=AX)
            hn = sp.tile([K, 1], F32, tag="hn")
            nc.vector.tensor_scalar(out=hn[:], in0=hnr[:], scalar1=0.0,
                                    scalar2=None, op0=mybir.AluOpType.is_gt)
            a_agg = cp.tile([K, 1], F32, tag="a_agg")
            nc.vector.tensor_scalar(out=a_agg[:], in0=hn[:], scalar1=0.5,
                                    scalar2=None, op0=mybir.AluOpType.mult)
            a_x = cp.tile([K, 1], F32, tag="a_x")
            nc.vector.tensor_scalar(out=a_x[:], in0=a_agg[:], scalar1=-1.0,
                                    scalar2=1.0, op0=mybir.AluOpType.mult,
                                    op1=mybir.AluOpType.add)

            p1t = load_c("p1t", [K, NPT])
            p2t = load_c("p2t", [K, NPT])
            g_m0 = sp.tile([P, 8, K], F32, tag="stg")
            nc.sync.dma_start(out=g_m0[:], in_=t_in["g_m"].rearrange(
                "(t p) k -> p t k", p=P))
            g_m = cp.tile([P, 8, K], F32R, tag="gm")
            nc.vector.tensor_copy(out=g_m[:], in_=g_m0[:])
            p2_m0 = sp.tile([P, 8, K], F32, tag="stg")
            nc.sync.dma_start(out=p2_m0[:], in_=t_in["p2_m"].rearrange(
                "(t p) k -> p t k", p=P))
            p2_m = cp.tile([P, 8, K], F32, tag="p2m")
            nc.vector.tensor_copy(out=p2_m[:], in_=p2_m0[:])
            gt_m = load_c("gt_m", [K, NPT])
            adj_r = cp.tile([K, K], F32R, tag="adjr")
            nc.vector.tensor_copy(out=adj_r[:], in_=adj[:])

            def explainer(x_in, wa4, wb4, b1r, w2full, b2v, lw4, lbr, tag):
                xT = transpose_k(x_in, tag + "xT")
                ps_a = mm_kd(xT, wa4, tag + "amm", bias=(ones_row, b1r))
                a_r = cp.tile([K, D], F32R, tag="xar")
                nc.vector.tensor_copy(out=a_r[:], in_=ps_a[:])
                ps_b = mm_kd(xT, wb4, tag + "bmm")
                b_r = cp.tile([K, D], F32R, tag="xbr")
                nc.vector.tensor_copy(out=b_r[:], in_=ps_b[:])

                ps_sums = pa.tile([K, 2], F32, tag="xsums")
                m_ts = []
                for t in range(8):
                    ps_h = pp.tile([P, D], F32, tag="mmps")
                    nc.tensor.matmul(out=ps_h[:], lhsT=p1t[:, t * P:(t + 1) * P],
                                     rhs=a_r[:], start=True, stop=False)
                    nc.tensor.matmul(out=ps_h[:], lhsT=p2t[:, t * P:(t + 1) * P],
                                     rhs=b_r[:], start=False, stop=True)
                    h_t = sp.tile([P, D], F32, tag="xht")
                    nc.scalar.activation(out=h_t[:], in_=ps_h[:],
                                         func=mybir.ActivationFunctionType.Relu)
                    hw = sp.tile([P, D], F32, tag="xhw")
                    nc.vector.tensor_mul(out=hw[:], in0=h_t[:], in1=w2full[:])
                    lgt = sp.tile([P, 1], F32, tag="xlgt")
                    nc.vector.reduce_sum(out=lgt[:], in_=hw[:], axis=AX)
                    nc.vector.tensor_scalar(out=lgt[:], in0=lgt[:],
                                            scalar1=float(b2v), scalar2=None,
                                            op0=mybir.AluOpType.add)
                    sg = sp.tile([P, 1], F32, tag="xsg")
                    nc.scalar.activation(out=sg[:], in_=lgt[:],
                                         func=mybir.ActivationFunctionType.Sigmoid)
                    ps_sel = pp.tile([P, K], F32, tag="selps")
                    nc.tensor.matmul(out=ps_sel[:], lhsT=p1t[:, t * P:(t + 1) * P],
                                     rhs=adj_r[:], start=True, stop=True)
                    selm = sp.tile([P, K], F32, tag="xselm")
                    nc.vector.tensor_mul(out=selm[:], in0=ps_sel[:],
                                         in1=p2_m[:, t, :])
                    adjf = sp.tile([P, 1], F32, tag="xadjf")
                    nc.vector.reduce_sum(out=adjf[:], in_=selm[:], axis=AX)
                    mkf = sp.tile([P, 1], F32, tag="xmkf")
                    nc.vector.tensor_scalar(out=mkf[:], in0=adjf[:], scalar1=0.0,
                                            scalar2=None, op0=mybir.AluOpType.is_gt)
                    m_t = cp.tile([P, 2], F32R, tag=f"xm{t}")
                    nc.vector.tensor_mul(out=m_t[:, :1], in0=sg[:], in1=mkf[:])
                    nc.vector.tensor_copy(out=m_t[:, 1:], in_=m_t[:, :1].bitcast(F32))
                    m_ts.append(m_t)
                    nc.tensor.matmul(out=ps_sums[:], lhsT=g_m[:, t, :],
                                     rhs=m_t[:], start=(t == 0), stop=(t == 7))
                inv = cp.tile([K, 2], F32R, tag="xinv")
                tmp = sp.tile([K, 1], F32, tag="xtmpi")
                nc.vector.tensor_scalar(out=tmp[:], in0=ps_sums[:, :1], scalar1=EPS,
                                        scalar2=None, op0=mybir.AluOpType.add)
                nc.vector.reciprocal(out=inv[:, :1], in_=tmp[:])
                nc.vector.tensor_copy(out=inv[:, 1:], in_=inv[:, :1].bitcast(F32))
                ps_agg = pa.tile([K, D], F32, tag="xagg")
                for t in range(8):
                    ps_ie = pp.tile([P, 2], F32, tag="selps")
                    nc.tensor.matmul(out=ps_ie[:], lhsT=gt_m[:, t * P:(t + 1) * P],
                                     rhs=inv[:], start=True, stop=True)
                    w_t = sp.tile([P, 1], F32, tag="xwt")
                    nc.vector.tensor_mul(out=w_t[:], in0=m_ts[t][:, :1].bitcast(F32),
                                         in1=ps_ie[:, :1])
                    ps_xe = pp.tile([P, D], F32, tag="mmps")
                    nc.tensor.matmul(out=ps_xe[:], lhsT=p2t[:, t * P:(t + 1) * P],
                                     rhs=x_in[:], start=True, stop=True)
                    wz = sp.tile([P, D], F32R, tag="xwz")
                    nc.vector.tensor_scalar_mul(out=wz[:], in0=ps_xe[:],
                                                scalar1=w_t[:])
                    nc.tensor.matmul(out=ps_agg[:], lhsT=g_m[:, t, :], rhs=wz[:],
                                     start=(t == 0), stop=(t == 7))
                emb = cp.tile([K, D], F32R, tag="xemb")
                t1 = sp.tile([K, D], F32, tag="xt1")
                nc.vector.tensor_scalar_mul(out=t1[:], in0=ps_agg[:],
                                            scalar1=a_agg[:])
                t2 = sp.tile([K, D], F32, tag="xt2")
                nc.vector.tensor_scalar_mul(out=t2[:], in0=x_in[:].bitcast(F32),
                                            scalar1=a_x[:])
                nc.vector.tensor_add(out=emb[:], in0=t1[:], in1=t2[:])
                embT = transpose_k(emb, tag + "embT")
                ps_hn = mm_kd(embT, lw4, tag + "hnmm", bias=(ones_row, lbr))
                h_next = cp.tile([K, D], F32R, tag=tag + "hnext")
                nc.vector.tensor_scalar_max(out=h_next[:], in0=ps_hn[:], scalar1=0.0)
                return h_next

            def w2_full(w2r, tag):
                ps_w = pp.tile([P, D], F32, tag="mmps")
                nc.tensor.matmul(out=ps_w[:], lhsT=ones_p1[:], rhs=w2r[:],
                                 start=True, stop=True)
                wf = cp.tile([P, D], F32, tag=tag)
                nc.vector.tensor_copy(out=wf[:], in_=ps_w[:])
                return wf

            w1a1 = load_c("w1a_1", [P, 4, D]); w1b1 = load_c("w1b_1", [P, 4, D])
            b11 = load_c("b1_1", [1, D]); w21 = load_c("w2_1", [1, D])
            lw1 = load_c("lw_1", [P, 4, D]); lb1 = load_c("lb_1", [1, D])
            w1a2 = load_c("w1a_2", [P, 4, D]); w1b2 = load_c("w1b_2", [P, 4, D])
            b12 = load_c("b1_2", [1, D]); w22 = load_c("w2_2", [1, D])
            lw2 = load_c("lw_2", [P, 4, D]); lb2 = load_c("lb_2", [1, D])
            wo4 = load_c("wout", [P, 4, P]); bo_r = load_c("bout", [1, P])
            w2f1 = w2_full(w21, "w2f1")
            w2f2 = w2_full(w22, "w2f2")

            h1 = explainer(z_cur, w1a1, w1b1, b11, w2f1, b2_1v, lw1, lb1, "e1")
            h2 = explainer(h1, w1a2, w1b2, b12, w2f2, b2_2v, lw2, lb2, "e2")
            h2T = transpose_k(h2, "h2T")
            ps_out = mm_kd(h2T, wo4, "omm", width=P, bias=(ones_row, bo_r))
            out_sb = sp.tile([K, P], F32, tag="outsb")
            nc.vector.tensor_copy(out=out_sb[:], in_=ps_out[:])
            nc.sync.dma_start(out=out_t[:, :], in_=out_sb[:])
    return nc


# ------------------------------------------------------------- host pipeline

_cache = {}


def kernel(x, edge_index, Wa, ba, Wp, bp,
           e1_W1, e1_b1, e1_W2, e1_b2, e1_lW, e1_lb,
           e2_W1, e2_b1, e2_W2, e2_b2, e2_lW, e2_lb,
           Wout, bout):
    x = np.asarray(x, np.float32)
    edge_index = np.asarray(edge_index)
    row = edge_index[0].astype(np.int64)
    col = edge_index[1].astype(np.int64)

    owner_r = row // SH_REAL
    owner_c = col // SH_REAL
    pos_r = owner_r * SH + (row - owner_r * SH_REAL)
    pos_c = owner_c * SH + (col - owner_c * SH_REAL)
    deg = np.bincount(row, minlength=N).astype(np.float32)

    xp = np.zeros((NCORE, SH, D), np.float32)
    degp = np.zeros((NCORE, SH, 1), np.float32)
    for m in range(NCORE):
        xp[m, :SH_REAL] = x[m * SH_REAL:(m + 1) * SH_REAL]
        degp[m, :SH_REAL, 0] = deg[m * SH_REAL:(m + 1) * SH_REAL]
    wa_in = np.asarray(Wa, np.float32)
    ba_in = np.asarray(ba, np.float32).reshape(1, K)
    if "p1" not in _cache:
        _cache["p1"] = SpmdRunner(build_p1())
    res1 = _cache["p1"]([dict(x=xp[m], wa=wa_in, ba=ba_in, deg=degp[m])
                         for m in range(NCORE)])

    s_full = np.concatenate([res1[m]["s_out"].reshape(SH, K)[:SH_REAL]
                             for m in range(NCORE)], axis=0)
    table = np.concatenate([res1[m]["sb_out"].reshape(SH, 32)
                            for m in range(NCORE)], axis=0)
    table_lines = np.zeros((NLINES, 128), ml_dtypes.bfloat16)
    table_lines[:NPOS // 4] = table.reshape(NPOS // 4, 128)

    cls = ((pos_r & 3) * 4 + (pos_c & 3)).astype(np.int64)
    key = owner_r * 16 + cls
    order = np.argsort(key, kind="stable")
    key_s = key[order]
    pr_s = (pos_r[order] >> 2).astype(np.int16)
    pc_s = (pos_c[order] >> 2).astype(np.int16)
    counts = np.bincount(key_s, minlength=NCORE * 16).reshape(NCORE, 16)
    caps = ((counts.max(axis=0) + 127) // 128) * 128
    caps = np.maximum(caps, 128)
    tot = int(caps.sum())
    pad_idx = np.int16(NPOS // 4)
    ir = np.full((NCORE, tot), pad_idx, np.int16)
    ic = np.full((NCORE, tot), pad_idx, np.int16)
    starts = np.concatenate([[0], np.cumsum(counts.reshape(-1))])[:-1].reshape(NCORE, 16)
    offs = np.concatenate([[0], np.cumsum(caps)])[:-1].astype(np.int64)
    for m in range(NCORE):
        for k in range(16):
            cnt = counts[m, k]
            s0 = starts[m, k]
            ir[m, offs[k]:offs[k] + cnt] = pr_s[s0:s0 + cnt]
            ic[m, offs[k]:offs[k] + cnt] = pc_s[s0:s0 + cnt]
    ir_w = np.stack([wrap_idxs(ir[m]) for m in range(NCORE)])
    ic_w = np.stack([wrap_idxs(ic[m]) for m in range(NCORE)])

    caps_t = tuple(int(c) for c in caps)
    ck = ("p2", caps_t)
    if ck not in _cache:
        _cache[ck] = SpmdRunner(build_p2(list(caps_t)))
    res2 = _cache[ck]([dict(tbl=table_lines, idx_r=ir_w[m], idx_c=ic_w[m])
                       for m in range(NCORE)])

    i_of = np.arange(NPT) // 32
    j_of = np.arange(NPT) % 32
    valid = (i_of < K) & (j_of < K)
    p1t = np.zeros((K, NPT), np.float32)
    p1t[np.minimum(i_of, K - 1), np.arange(NPT)] = valid.astype(np.float32)
    p2t = np.zeros((K, NPT), np.float32)
    p2t[np.minimum(j_of, K - 1), np.arange(NPT)] = valid.astype(np.float32)
    g_m = np.zeros((NPT, K), np.float32)
    g_m[np.arange(NPT), np.minimum(i_of, K - 1)] = valid.astype(np.float32)
    gt_m = np.ascontiguousarray(g_m.T)
    p2_m = np.zeros((NPT, K), np.float32)
    p2_m[np.arange(NPT), np.minimum(j_of, K - 1)] = valid.astype(np.float32)

    f32 = lambda a: np.ascontiguousarray(np.asarray(a, np.float32))
    in3 = dict(
        zxp=np.concatenate([res1[m]["zx_out"] for m in range(NCORE)], 0),
        ssp=np.concatenate([res1[m]["ss_out"] for m in range(NCORE)], 0),
        adjp=np.concatenate([res2[m]["adj_out"] for m in range(NCORE)], 0),
        volp=np.concatenate([res1[m]["vol_out"] for m in range(NCORE)], 0),
        wp=f32(Wp), bp=f32(bp).reshape(1, D),
        w1a_1=f32(np.asarray(e1_W1)[:D]), w1b_1=f32(np.asarray(e1_W1)[D:]),
        b1_1=f32(e1_b1).reshape(1, D), w2_1=f32(e1_W2).reshape(1, D),
        lw_1=f32(e1_lW), lb_1=f32(e1_lb).reshape(1, D),
        w1a_2=f32(np.asarray(e2_W1)[:D]), w1b_2=f32(np.asarray(e2_W1)[D:]),
        b1_2=f32(e2_b1).reshape(1, D), w2_2=f32(e2_W2).reshape(1, D),
        lw_2=f32(e2_lW), lb_2=f32(e2_lb).reshape(1, D),
        wout=f32(Wout), bout=f32(bout).reshape(1, P),
        p1t=p1t, p2t=p2t, g_m=g_m, gt_m=gt_m, p2_m=p2_m,
        eye=np.eye(K, dtype=np.float32),
    )
    b2_1v = float(np.asarray(e1_b2).reshape(-1)[0])
    b2_2v = float(np.asarray(e2_b2).reshape(-1)[0])
    pk = ("p3", round(b2_1v, 9), round(b2_2v, 9))
    if pk not in _cache:
        _cache[pk] = SpmdRunner(build_p3(b2_1v, b2_2v))
    res3 = _cache[pk]([in3] * NCORE)
    out = res3[0]["out"]
    mincut = np.float32(res3[0]["mincut"][0, 0])
    ortho = np.float32(res3[0]["ortho"][0, 0])
    Z = res3[0]["z"]
    return (out, mincut, ortho, Z, s_full)


# revision 16
# speedup vs baseline: 25.7216x; 1.0281x over previous
"""Trainium2 Bass kernel for nn_MinCutExplainerGNN (8 NeuronCores, SPMD).

Structure (3 SPMD launches, no collectives):
  P1 (x-phase, nodes sharded 8-way): S = softmax(x@Wa+ba); partials
     ZX = S^T x, SSaug = S^T [S|1], vol = sum(deg * rowsum(S)); S fp32 out +
     bf16 gather table shard.
  host: concatenates the bf16 S table (pure data movement), edge index
     preprocessing (integer only: bucketing by destination shard, class split
     by (pos_r&3, pos_c&3) for 256B-line gather addressing, padding).
  P2 (edge phase, edges sharded by destination): two-sided dma_gather of S
     rows (64B bf16 payloads out of 256B-stride lines), adj partial
     accumulation via PSUM outer-product matmuls.
  P3 (replicated finisher): reduce partials, Z = ZX@Wp + colsum*bp, losses,
     the two dense-K explainer layers, final linear.
"""
import numpy as np
import ml_dtypes
import jax
from jax.sharding import Mesh, PartitionSpec
from jax.experimental.shard_map import shard_map

import concourse.bacc as bacc
import concourse.bass as bass
import concourse.mybir as mybir
import concourse.tile as tile
from concourse import ap_utils
from concourse.bass import MemorySpace
from concourse._compat import exact_div
from concourse import bass2jax
from concourse.bass2jax import _bass_exec_p, partition_id_tensor
from concourse.masks import make_identity

P = 128
NCORE = 8
N = 100000
D = 512
K = 30
EPS = 1e-9
SH_REAL = 12500            # real rows per shard
SH = 12544                 # padded rows per shard (98 * 128)
TILES = SH // P            # 98
NPOS = NCORE * SH          # 100352
NLINES = NPOS // 4 + 128   # table lines (4 rows/256B line) + zero pad lines
F32, F32R, BF16, I16 = (mybir.dt.float32, mybir.dt.float32r,
                        mybir.dt.bfloat16, mybir.dt.int16)
AX = mybir.AxisListType.X
CHUNK = 16128              # idxs per dma_gather instruction (scratch cap ~16200)


# ---------------------------------------------------------------- utilities

def dma_gather_raw(gp, out_ap, in_ap, idxs_ap, num_idxs, elem_size, elem_step,
                   queue_num=0):
    gp._assert_queue_num(queue_num)
    assert idxs_ap.dtype == I16
    assert in_ap.space == MemorySpace.DRAM
    assert in_ap.dtype == out_ap.dtype
    assert num_idxs % 128 == 0
    assert ap_utils.ap_is_contiguous(out_ap.ap[1:])
    assert ap_utils.ap_is_contiguous(idxs_ap.ap[1:])
    assert in_ap.ap[-1][1] == out_ap.ap[-1][1] == elem_size
    assert out_ap.ap[0][1] * out_ap.ap[1][1] == num_idxs
    assert in_ap.ap[0][0] == elem_step
    stride_bytes_256 = exact_div(elem_step * mybir.dt.size(in_ap.dtype), 256)
    _in_ap = gp.lower_ap_dma(in_ap, for_custom_bir_dma=True)
    return gp.add_instruction(
        mybir.InstDMAGatherAnt(
            name=gp.bass.get_next_instruction_name(),
            ins=[*_in_ap, gp.lower_ap(idxs_ap),
                 gp.lower_val_access(gp.to_reg(num_idxs))],
            outs=[gp.lower_ap(out_ap)],
            transpose=False, num_idxs=num_idxs, elem_size=elem_size,
            stride_bytes_256=stride_bytes_256, gen_mode=0, single_packet=False,
            queue_num=queue_num, sbuf_tokens_per_rank=0,
            sbuf_free_dim_per_rank=0, sbuf_free_dim_pad_per_rank=0,
            sbuf_byte_offset=0,
        ))


def wrap_idxs(idx):
    n = idx.shape[0]
    w = idx.astype(np.int16).reshape(n // 16, 16).T
    return np.tile(w, (8, 1))


class SpmdRunner:
    def __init__(self, nc, n_cores=NCORE):
        if not nc.is_finalized():
            nc.finalize()
        self.nc, self.n_cores = nc, n_cores
        bass2jax.install_neuronx_cc_hook()
        pname = nc.partition_id_tensor.name if nc.partition_id_tensor else None
        in_names, out_names, out_avals, zouts = [], [], [], []
        for alloc in nc.m.functions[0].allocations:
            if not isinstance(alloc, mybir.MemoryLocationSet):
                continue
            name = alloc.memorylocations[0].name
            if alloc.kind == "ExternalInput":
                if name != pname:
                    in_names.append(name)
            elif alloc.kind == "ExternalOutput":
                shape, dtype = tuple(alloc.tensor_shape), mybir.dt.np(alloc.dtype)
                out_names.append(name)
                out_avals.append(jax.core.ShapedArray(shape, dtype))
                zouts.append((shape, dtype))
        self.in_names, self.out_names = in_names, out_names
        self.out_avals, self.zero_outs = out_avals, zouts
        n_params, n_outs = len(in_names), len(out_avals)
        all_in = list(in_names) + list(out_names) + ([pname] if pname else [])

        def _body(*args):
            operands = list(args)
            if pname is not None:
                operands.append(partition_id_tensor())
            return tuple(_bass_exec_p.bind(
                *operands, out_avals=tuple(out_avals), in_names=tuple(all_in),
                out_names=tuple(out_names), lowering_input_output_aliases=(),
                sim_require_finite=True, sim_require_nnan=True, nc=nc))

        devices = jax.devices()[:n_cores]
        mesh = Mesh(np.asarray(devices), ("core",))
        self.sharded = jax.jit(
            shard_map(_body, mesh=mesh,
                      in_specs=(PartitionSpec("core"),) * (n_params + n_outs),
                      out_specs=(PartitionSpec("core"),) * n_outs,
                      check_rep=False),
            donate_argnums=tuple(range(n_params, n_params + n_outs)),
            keep_unused=True)

    def __call__(self, in_maps):
        concat_in = [np.concatenate([np.asarray(in_maps[c][k])
                                     for c in range(self.n_cores)], axis=0)
                     for k in self.in_names]
        zeros = [np.zeros((self.n_cores * s[0], *s[1:]), d)
                 for (s, d) in self.zero_outs]
        outs = self.sharded(*concat_in, *zeros)
        jax.block_until_ready(outs)
        return [{k: np.asarray(outs[i]).reshape(self.n_cores,
                                                *self.out_avals[i].shape)[c]
                 for i, k in enumerate(self.out_names)}
                for c in range(self.n_cores)]


# ------------------------------------------------------------------ P1 build

def build_p1():
    nc = bacc.Bacc(None, num_devices=NCORE)
    x_t = nc.dram_tensor("x", [SH, D], F32, kind="ExternalInput")
    wa_t = nc.dram_tensor("wa", [D, K], F32, kind="ExternalInput")
    ba_t = nc.dram_tensor("ba", [1, K], F32, kind="ExternalInput")
    deg_t = nc.dram_tensor("deg", [SH, 1], F32, kind="ExternalInput")
    s_out = nc.dram_tensor("s_out", [TILES, P, K], F32, kind="ExternalOutput")
    sb_out = nc.dram_tensor("sb_out", [TILES, P, 32], BF16, kind="ExternalOutput")
    zx_out = nc.dram_tensor("zx_out", [K, D], F32, kind="ExternalOutput")
    ss_out = nc.dram_tensor("ss_out", [K, K + 2], F32, kind="ExternalOutput")
    vol_out = nc.dram_tensor("vol_out", [1, 1], F32, kind="ExternalOutput")

    with nc.allow_low_precision(reason="float32r operands"), \
         tile.TileContext(nc) as tc:
        with (tc.tile_pool(name="consts", bufs=1) as cp,
              tc.tile_pool(name="sbuf", bufs=3) as sp,
              tc.tile_pool(name="psum", bufs=2, space="PSUM") as pp,
              tc.tile_pool(name="psacc", bufs=1, space="PSUM") as pa):
            ident0 = cp.tile([P, P], F32)
            make_identity(nc, ident0[:])
            ident = cp.tile([P, P], F32R)
            nc.vector.tensor_copy(out=ident[:], in_=ident0[:])
            wa0 = cp.tile([P, 4, K], F32)
            nc.sync.dma_start(out=wa0[:], in_=wa_t.rearrange("(c p) k -> p c k", p=P))
            wa = cp.tile([P, 4, K], F32R)
            nc.vector.tensor_copy(out=wa[:], in_=wa0[:])
            ones1f = cp.tile([1, P], F32)
            nc.vector.memset(ones1f[:], 1.0)
            ones1 = cp.tile([1, P], F32R)
            nc.vector.tensor_copy(out=ones1[:], in_=ones1f[:])
            ba0 = cp.tile([1, K], F32)
            nc.sync.dma_start(out=ba0[:], in_=ba_t[:, :])
            ba = cp.tile([1, K], F32R)
            nc.vector.tensor_copy(out=ba[:], in_=ba0[:])
            vol_acc = cp.tile([P, 1], F32)
            nc.vector.memset(vol_acc[:], 0.0)
            ones_col = cp.tile([P, 1], F32)
            nc.vector.memset(ones_col[:], 1.0)
            zero_col = cp.tile([P, 1], F32)
            nc.vector.memset(zero_col[:], 0.0)
            valid = cp.tile([P, 1], F32)
            nc.vector.memset(valid[:], 1.0)
            nc.gpsimd.affine_select(out=valid[:], in_=valid[:],
                                    compare_op=mybir.AluOpType.is_ge,
                                    fill=0.0, base=SH_REAL - 97 * P - 1,
                                    pattern=[[0, 1]], channel_multiplier=-1)
            ps_zx = pa.tile([K, D], F32, tag="zx")
            ps_ss = pa.tile([K, K + 2], F32, tag="ss")

            for t in range(TILES):
                xt0 = sp.tile([P, D], F32, tag="x0")
                nc.sync.dma_start(out=xt0[:], in_=x_t[t * P:(t + 1) * P, :])
                xt = sp.tile([P, D], F32R, tag="x")
                nc.vector.tensor_copy(out=xt[:], in_=xt0[:])
                xT = sp.tile([P, 4, P], F32R, tag="xT")
                for c in range(4):
                    tp = pp.tile([P, P], F32R, tag="tp")
                    nc.tensor.transpose(out=tp[:], in_=xt[:, c * P:(c + 1) * P],
                                        identity=ident[:])
                    nc.vector.tensor_copy(out=xT[:, c, :], in_=tp[:])
                # logits [P, K] = x @ Wa + ba
                lg = pp.tile([P, K], F32, tag="lg")
                for c in range(4):
                    nc.tensor.matmul(out=lg[:], lhsT=xT[:, c, :], rhs=wa[:, c, :],
                                     start=(c == 0), stop=False)
                nc.tensor.matmul(out=lg[:], lhsT=ones1[:], rhs=ba[:],
                                 start=False, stop=True)
                ex = sp.tile([P, K], F32, tag="ex")
                nc.scalar.activation(out=ex[:], in_=lg[:],
                                     func=mybir.ActivationFunctionType.Exp)
                ssum = sp.tile([P, 1], F32, tag="ssum")
                nc.vector.reduce_sum(out=ssum[:], in_=ex[:], axis=AX)
                rcp = sp.tile([P, 1], F32, tag="rcp")
                nc.vector.reciprocal(out=rcp[:], in_=ssum[:])
                s_f = sp.tile([P, K], F32R, tag="sf")
                nc.vector.tensor_scalar_mul(out=s_f[:], in0=ex[:], scalar1=rcp[:])
                if t == TILES - 1:
                    # zero padding rows 12500..12543 (partitions 84..127)
                    nc.vector.tensor_scalar_mul(out=s_f[:], in0=s_f[:].bitcast(F32),
                                                scalar1=valid[:])
                # outputs: S fp32 + bf16 table shard
                nc.sync.dma_start(out=s_out[t, :, :], in_=s_f[:].bitcast(F32))
                s_b = sp.tile([P, 32], BF16, tag="sb")
                nc.vector.memset(s_b[:, K:], 0.0)
                nc.vector.tensor_copy(out=s_b[:, :K], in_=s_f[:])
                nc.sync.dma_start(out=sb_out[t, :, :], in_=s_b[:])
                # ZX += S^T x ; SSaug += S^T [S|1]
                nc.tensor.matmul(out=ps_zx[:], lhsT=s_f[:], rhs=xt[:],
                                 start=(t == 0), stop=(t == TILES - 1))
                s_aug = sp.tile([P, K + 2], F32R, tag="saug")
                nc.vector.tensor_copy(out=s_aug[:, :K], in_=s_f[:])
                nc.vector.tensor_copy(out=s_aug[:, K + 1:], in_=zero_col[:])
                if t == TILES - 1:
                    nc.vector.tensor_copy(out=s_aug[:, K:K + 1], in_=valid[:])
                else:
                    nc.vector.tensor_copy(out=s_aug[:, K:K + 1], in_=ones_col[:])
                nc.tensor.matmul(out=ps_ss[:], lhsT=s_f[:], rhs=s_aug[:],
                                 start=(t == 0), stop=(t == TILES - 1))
                # vol partial: sum(deg * rowsum(S))
                dg = sp.tile([P, 1], F32, tag="dg")
                nc.sync.dma_start(out=dg[:], in_=deg_t[t * P:(t + 1) * P, :])
                rs = sp.tile([P, 1], F32, tag="rs")
                nc.vector.reduce_sum(out=rs[:], in_=s_f[:].bitcast(F32), axis=AX)
                dv = sp.tile([P, 1], F32, tag="dv")
                nc.vector.tensor_mul(out=dv[:], in0=rs[:], in1=dg[:])
                nc.vector.tensor_add(out=vol_acc[:], in0=vol_acc[:], in1=dv[:])

            zx_sb = sp.tile([K, D], F32)
            nc.vector.tensor_copy(out=zx_sb[:], in_=ps_zx[:])
            nc.sync.dma_start(out=zx_out[:, :], in_=zx_sb[:])
            ss_sb = sp.tile([K, K + 2], F32)
            nc.vector.tensor_copy(out=ss_sb[:], in_=ps_ss[:])
            nc.sync.dma_start(out=ss_out[:, :], in_=ss_sb[:])
            # vol: partition-reduce via matmul with ones
            va_r = cp.tile([P, 1], F32R)
            nc.vector.tensor_copy(out=va_r[:], in_=vol_acc[:])
            onesp = cp.tile([P, 2], F32R)
            nc.vector.tensor_copy(out=onesp[:, :1], in_=ones_col[:])
            nc.vector.tensor_copy(out=onesp[:, 1:], in_=ones_col[:])
            ps_v = pp.tile([1, 2], F32, tag="v")
            nc.tensor.matmul(out=ps_v[:], lhsT=va_r[:], rhs=onesp[:],
                             start=True, stop=True)
            v_sb = sp.tile([1, 1], F32)
            nc.vector.tensor_copy(out=v_sb[:], in_=ps_v[:, :1])
            nc.sync.dma_start(out=vol_out[:, :], in_=v_sb[:])
    return nc


# ------------------------------------------------------------------ P2 build

def build_p2(class_caps):
    """class_caps: list of 16 ints (multiples of 128), same for all cores."""
    tot = sum(class_caps)
    nc = bacc.Bacc(None, num_devices=NCORE, num_swdge_queues=4)
    tbl_t = nc.dram_tensor("tbl", [NLINES, 128], BF16, kind="ExternalInput")
    ir_t = nc.dram_tensor("idx_r", [16, tot // 16], I16, kind="ExternalInput")
    ic_t = nc.dram_tensor("idx_c", [16, tot // 16], I16, kind="ExternalInput")
    adj_out = nc.dram_tensor("adj_out", [K, K], F32, kind="ExternalOutput")

    with tile.TileContext(nc) as tc:
        with (tc.tile_pool(name="sbuf", bufs=2) as sp,
              tc.tile_pool(name="idx", bufs=2) as ip,
              tc.tile_pool(name="psum", bufs=1, space="PSUM") as pp):
            ps_adj = pp.tile([K, K], F32, tag="adj")
            first = True
            chunks = []
            off = 0
            for k, cap in enumerate(class_caps):
                o = 0
                while o < cap:
                    n = min(CHUNK, cap - o)
                    chunks.append((k, off + o, n))
                    o += n
                off += cap
            last_i = len(chunks) - 1
            for ci, (k, start, n) in enumerate(chunks):
                q = ci % 4
                kr, kc = (k >> 2) & 3, k & 3
                itr = ip.tile([P, n // 16], I16, tag=f"ir{q}")
                itc = ip.tile([P, n // 16], I16, tag=f"ic{q}")
                for g in range(8):
                    nc.sync.dma_start(
                        out=itr[16 * g:16 * (g + 1), :],
                        in_=ir_t[:, start // 16:(start + n) // 16])
                    nc.sync.dma_start(
                        out=itc[16 * g:16 * (g + 1), :],
                        in_=ic_t[:, start // 16:(start + n) // 16])
                gr = sp.tile([P, n // P, 32], BF16, tag=f"gr{q}")
                dma_gather_raw(nc.gpsimd, gr[:], tbl_t[:, 32 * kr:32 * kr + 32],
                               itr[:], n, 32, 128, queue_num=q)
                gc = sp.tile([P, n // P, 32], BF16, tag=f"gc{q}")
                dma_gather_raw(nc.gpsimd, gc[:], tbl_t[:, 32 * kc:32 * kc + 32],
                               itc[:], n, 32, 128, queue_num=q)
                for g in range(n // P):
                    nc.tensor.matmul(out=ps_adj[:], lhsT=gr[:, g, :K],
                                     rhs=gc[:, g, :K], start=first,
                                     stop=(ci == last_i and g == n // P - 1))
                    first = False
            adj_sb = sp.tile([K, K], F32, tag="adjsb")
            nc.vector.tensor_copy(out=adj_sb[:], in_=ps_adj[:])
            nc.sync.dma_start(out=adj_out[:, :], in_=adj_sb[:])
    return nc


# ------------------------------------------------------------------ P3 build

NPT = 1024                 # padded pair-rows (i*32+j, j padded to 32) -> 8 tiles


def build_p3(b2_1v, b2_2v):
    nc = bacc.Bacc(None, num_devices=NCORE)
    t_in = {}
    for nm, shp in (("zxp", [NCORE * K, D]), ("ssp", [NCORE * K, K + 2]),
                    ("adjp", [NCORE * K, K]), ("volp", [NCORE, 1]),
                    ("wp", [D, D]), ("bp", [1, D]),
                    ("w1a_1", [D, D]), ("w1b_1", [D, D]), ("b1_1", [1, D]),
                    ("w2_1", [1, D]), ("lw_1", [D, D]), ("lb_1", [1, D]),
                    ("w1a_2", [D, D]), ("w1b_2", [D, D]), ("b1_2", [1, D]),
                    ("w2_2", [1, D]), ("lw_2", [D, D]), ("lb_2", [1, D]),
                    ("wout", [D, P]), ("bout", [1, P]),
                    ("p1t", [K, NPT]), ("p2t", [K, NPT]),
                    ("g_m", [NPT, K]), ("gt_m", [K, NPT]), ("p2_m", [NPT, K]),
                    ("eye", [K, K])):
        t_in[nm] = nc.dram_tensor(nm, shp, F32, kind="ExternalInput")
    out_t = nc.dram_tensor("out", [K, P], F32, kind="ExternalOutput")
    mc_t = nc.dram_tensor("mincut", [1, 1], F32, kind="ExternalOutput")
    or_t = nc.dram_tensor("ortho", [1, 1], F32, kind="ExternalOutput")
    z_t = nc.dram_tensor("z", [K, D], F32, kind="ExternalOutput")

    with nc.allow_low_precision(reason="float32r operands"), \
         tile.TileContext(nc) as tc:
        with (tc.tile_pool(name="consts", bufs=1) as cp,
              tc.tile_pool(name="sbuf", bufs=1) as sp,
              tc.tile_pool(name="psum", bufs=1, space="PSUM") as pp,
              tc.tile_pool(name="psacc", bufs=1, space="PSUM") as pa):
            ident0 = cp.tile([P, P], F32)
            make_identity(nc, ident0[:])
            ident = cp.tile([P, P], F32R)
            nc.vector.tensor_copy(out=ident[:], in_=ident0[:])

            def load_c(nm, shape, dt=F32R, tag=None):
                t0 = sp.tile(shape, F32, tag="stg")
                src = t_in[nm]
                if len(shape) == 3:
                    nc.sync.dma_start(out=t0[:], in_=src.rearrange(
                        "(c p) n -> p c n", p=shape[0]))
                else:
                    nc.sync.dma_start(out=t0[:], in_=src[:, :])
                t1 = cp.tile(shape, dt, tag=(tag or nm))
                nc.vector.tensor_copy(out=t1[:], in_=t0[:])
                return t1

            def reduce_parts(nm, cols, tag):
                t0 = sp.tile([K, NCORE, cols], F32, tag=tag + "0")
                nc.sync.dma_start(out=t0[:], in_=t_in[nm].rearrange(
                    "(c k) n -> k c n", k=K))
                acc = cp.tile([K, cols], F32, tag=tag)
                nc.vector.tensor_copy(out=acc[:], in_=t0[:, 0, :])
                for c in range(1, NCORE):
                    nc.vector.tensor_add(out=acc[:], in0=acc[:], in1=t0[:, c, :])
                return acc

            zx = reduce_parts("zxp", D, "zx")           # [K, D]
            ssa = reduce_parts("ssp", K + 2, "ssa")     # [K, K+1]
            adj = reduce_parts("adjp", K, "adj")        # [K, K]
            volp0 = sp.tile([1, NCORE], F32, tag="volp0")
            nc.sync.dma_start(out=volp0[:], in_=t_in["volp"].rearrange(
                "(c o) n -> o (c n)", o=1))
            vol = cp.tile([1, 1], F32, tag="vol")
            nc.vector.reduce_sum(out=vol[:], in_=volp0[:], axis=AX)

            ones_f = cp.tile([P, 1], F32)
            nc.vector.memset(ones_f[:], 1.0)
            ones_fr = cp.tile([1, P], F32)
            nc.vector.memset(ones_fr[:], 1.0)
            ones_k = cp.tile([K, 2], F32R)
            nc.vector.tensor_copy(out=ones_k[:, :1], in_=ones_f[:K, :])
            nc.vector.tensor_copy(out=ones_k[:, 1:], in_=ones_f[:K, :])
            ones_row = cp.tile([1, K], F32R)
            nc.vector.tensor_copy(out=ones_row[:], in_=ones_fr[:, :K])
            ones_p1 = cp.tile([1, P], F32R)
            nc.vector.tensor_copy(out=ones_p1[:], in_=ones_fr[:])

            def transpose_k(src_r, tag):
                dst = cp.tile([P, 4, K], F32R, tag=tag)
                for c in range(4):
                    tp = pp.tile([P, K], F32R, tag="tpk")
                    nc.tensor.transpose(out=tp[:], in_=src_r[:, c * P:(c + 1) * P],
                                        identity=ident[:K, :K])
                    nc.vector.tensor_copy(out=dst[:, c, :], in_=tp[:])
                return dst

            def mm_kd(xT, w4, tag, width=D, bias=None):
                ps = pp.tile([K, width], F32, tag="mmps")
                for c in range(4):
                    nc.tensor.matmul(out=ps[:], lhsT=xT[:, c, :],
                                     rhs=w4[:, c, :width],
                                     start=(c == 0),
                                     stop=(c == 3 and bias is None))
                if bias is not None:
                    blhs, brhs = bias
                    nc.tensor.matmul(out=ps[:], lhsT=blhs[:], rhs=brhs[:, :width],
                                     start=False, stop=True)
                return ps

            wp4 = load_c("wp", [P, 4, D])
            bp_r = load_c("bp", [1, D])
            zx_r = cp.tile([K, D], F32R, tag="zxr")
            nc.vector.tensor_copy(out=zx_r[:], in_=zx[:])
            zxT = transpose_k(zx_r, "zxT")
            colsum = cp.tile([K, 2], F32R, tag="colsum")
            nc.vector.tensor_copy(out=colsum[:, :1], in_=ssa[:, K:K + 1])
            nc.vector.tensor_copy(out=colsum[:, 1:], in_=ssa[:, K:K + 1])
            ps_ct = pp.tile([2, K], F32R, tag="selps")
            nc.tensor.transpose(out=ps_ct[:], in_=colsum[:], identity=ident[:K, :K])
            colsumT = cp.tile([1, K], F32R, tag="colsumT")
            nc.vector.tensor_copy(out=colsumT[:], in_=ps_ct[:1, :])
            ps_z = mm_kd(zxT, wp4, "zmm", bias=(colsumT, bp_r))
            z_cur = cp.tile([K, D], F32R, tag="zcur")
            nc.vector.tensor_copy(out=z_cur[:], in_=ps_z[:])
            z_sb = sp.tile([K, D], F32, tag="zsb")
            nc.vector.tensor_copy(out=z_sb[:], in_=z_cur[:])
            nc.sync.dma_start(out=z_t[:, :], in_=z_sb[:])

            # losses
            eye_r = load_c("eye", [K, K], dt=F32)
            dif = sp.tile([K, K], F32, tag="dif")
            nc.vector.tensor_sub(out=dif[:], in0=ssa[:, :K], in1=eye_r[:])
            sq = sp.tile([K, K], F32, tag="sq")
            nc.vector.tensor_mul(out=sq[:], in0=dif[:], in1=dif[:])
            rs = sp.tile([K, 1], F32R, tag="rs3")
            nc.vector.reduce_sum(out=rs[:], in_=sq[:], axis=AX)
            ps_o = pp.tile([1, 2], F32, tag="tiny1")
            nc.tensor.matmul(out=ps_o[:], lhsT=rs[:], rhs=ones_k[:],
                             start=True, stop=True)
            orto = sp.tile([1, 1], F32, tag="orto")
            nc.scalar.activation(out=orto[:], in_=ps_o[:, :1],
                                 func=mybir.ActivationFunctionType.Sqrt)
            nc.sync.dma_start(out=or_t[:, :], in_=orto[:])

            diag = sp.tile([K, K], F32, tag="diag")
            nc.vector.tensor_mul(out=diag[:], in0=adj[:], in1=eye_r[:])
            trs = sp.tile([K, 1], F32R, tag="trs")
            nc.vector.reduce_sum(out=trs[:], in_=diag[:], axis=AX)
            ps_c = pp.tile([1, 2], F32, tag="tiny1")
            nc.tensor.matmul(out=ps_c[:], lhsT=trs[:], rhs=ones_k[:],
                             start=True, stop=True)
            den = sp.tile([1, 1], F32, tag="den")
            nc.vector.tensor_scalar(out=den[:], in0=vol[:], scalar1=EPS,
                                    scalar2=None, op0=mybir.AluOpType.add)
            rden = sp.tile([1, 1], F32, tag="rden")
            nc.vector.reciprocal(out=rden[:], in_=den[:])
            mcv = sp.tile([1, 1], F32, tag="mcv")
            nc.vector.tensor_mul(out=mcv[:], in0=ps_c[:, :1], in1=rden[:])
            nc.vector.tensor_scalar(out=mcv[:], in0=mcv[:], scalar1=-1.0,
                                    scalar2=None, op0=mybir.AluOpType.mult)
            nc.sync.dma_start(out=mc_t[:, :], in_=mcv[:])

            # mask / alphas
            mask = cp.tile([K, K], F32, tag="mask")
            nc.vector.tensor_scalar(out=mask[:], in0=adj[:], scalar1=0.0,
                                    scalar2=None, op0=mybir.AluOpType.is_gt)
            hnr = sp.tile([K, 1], F32, tag="hnr")
            nc.vector.reduce_sum(out=hnr[:], in_=mask[:], axis=AX)
            hn = sp.tile([K, 1], F32, tag="hn")
            nc.vector.tensor_scalar(out=hn[:], in0=hnr[:], scalar1=0.0,
                                    scalar2=None, op0=mybir.AluOpType.is_gt)
            a_agg = cp.tile([K, 1], F32, tag="a_agg")
            nc.vector.tensor_scalar(out=a_agg[:], in0=hn[:], scalar1=0.5,
                                    scalar2=None, op0=mybir.AluOpType.mult)
            a_x = cp.tile([K, 1], F32, tag="a_x")
            nc.vector.tensor_scalar(out=a_x[:], in0=a_agg[:], scalar1=-1.0,
                                    scalar2=1.0, op0=mybir.AluOpType.mult,
                                    op1=mybir.AluOpType.add)

            p1t = load_c("p1t", [K, NPT])
            p2t = load_c("p2t", [K, NPT])
            g_m0 = sp.tile([P, 8, K], F32, tag="stg")
            nc.sync.dma_start(out=g_m0[:], in_=t_in["g_m"].rearrange(
                "(t p) k -> p t k", p=P))
            g_m = cp.tile([P, 8, K], F32R, tag="gm")
            nc.vector.tensor_copy(out=g_m[:], in_=g_m0[:])
            p2_m0 = sp.tile([P, 8, K], F32, tag="stg")
            nc.sync.dma_start(out=p2_m0[:], in_=t_in["p2_m"].rearrange(
                "(t p) k -> p t k", p=P))
            p2_m = cp.tile([P, 8, K], F32, tag="p2m")
            nc.vector.tensor_copy(out=p2_m[:], in_=p2_m0[:])
            gt_m = load_c("gt_m", [K, NPT])
            adj_r = cp.tile([K, K], F32R, tag="adjr")
            nc.vector.tensor_copy(out=adj_r[:], in_=adj[:])

            def explainer(x_in, wa4, wb4, b1r, w2full, b2v, lw4, lbr, tag):
                xT = transpose_k(x_in, tag + "xT")
                ps_a = mm_kd(xT, wa4, tag + "amm", bias=(ones_row, b1r))
                a_r = cp.tile([K, D], F32R, tag="xar")
                nc.vector.tensor_copy(out=a_r[:], in_=ps_a[:])
                ps_b = mm_kd(xT, wb4, tag + "bmm")
                b_r = cp.tile([K, D], F32R, tag="xbr")
                nc.vector.tensor_copy(out=b_r[:], in_=ps_b[:])

                ps_sums = pa.tile([K, 2], F32, tag="xsums")
                m_ts = []
                for t in range(8):
                    ps_h = pp.tile([P, D], F32, tag="mmps")
                    nc.tensor.matmul(out=ps_h[:], lhsT=p1t[:, t * P:(t + 1) * P],
                                     rhs=a_r[:], start=True, stop=False)
                    nc.tensor.matmul(out=ps_h[:], lhsT=p2t[:, t * P:(t + 1) * P],
                                     rhs=b_r[:], start=False, stop=True)
                    h_t = sp.tile([P, D], F32, tag="xht")
                    nc.scalar.activation(out=h_t[:], in_=ps_h[:],
                                         func=mybir.ActivationFunctionType.Relu)
                    hw = sp.tile([P, D], F32, tag="xhw")
                    nc.vector.tensor_mul(out=hw[:], in0=h_t[:], in1=w2full[:])
                    lgt = sp.tile([P, 1], F32, tag="xlgt")
                    nc.vector.reduce_sum(out=lgt[:], in_=hw[:], axis=AX)
                    nc.vector.tensor_scalar(out=lgt[:], in0=lgt[:],
                                            scalar1=float(b2v), scalar2=None,
                                            op0=mybir.AluOpType.add)
                    sg = sp.tile([P, 1], F32, tag="xsg")
                    nc.scalar.activation(out=sg[:], in_=lgt[:],
                                         func=mybir.ActivationFunctionType.Sigmoid)
                    ps_sel = pp.tile([P, K], F32, tag="selps")
                    nc.tensor.matmul(out=ps_sel[:], lhsT=p1t[:, t * P:(t + 1) * P],
                                     rhs=adj_r[:], start=True, stop=True)
                    selm = sp.tile([P, K], F32, tag="xselm")
                    nc.vector.tensor_mul(out=selm[:], in0=ps_sel[:],
                                         in1=p2_m[:, t, :])
                    adjf = sp.tile([P, 1], F32, tag="xadjf")
                    nc.vector.reduce_sum(out=adjf[:], in_=selm[:], axis=AX)
                    mkf = sp.tile([P, 1], F32, tag="xmkf")
                    nc.vector.tensor_scalar(out=mkf[:], in0=adjf[:], scalar1=0.0,
                                            scalar2=None, op0=mybir.AluOpType.is_gt)
                    m_t = cp.tile([P, 2], F32R, tag=f"xm{t}")
                    nc.vector.tensor_mul(out=m_t[:, :1], in0=sg[:], in1=mkf[:])
                    nc.vector.tensor_copy(out=m_t[:, 1:], in_=m_t[:, :1].bitcast(F32))
                    m_ts.append(m_t)
                    nc.tensor.matmul(out=ps_sums[:], lhsT=g_m[:, t, :],
                                     rhs=m_t[:], start=(t == 0), stop=(t == 7))
                inv = cp.tile([K, 2], F32R, tag="xinv")
                tmp = sp.tile([K, 1], F32, tag="xtmpi")
                nc.vector.tensor_scalar(out=tmp[:], in0=ps_sums[:, :1], scalar1=EPS,
                                        scalar2=None, op0=mybir.AluOpType.add)
                nc.vector.reciprocal(out=inv[:, :1], in_=tmp[:])
                nc.vector.tensor_copy(out=inv[:, 1:], in_=inv[:, :1].bitcast(F32))
                ps_agg = pa.tile([K, D], F32, tag="xagg")
                for t in range(8):
                    ps_ie = pp.tile([P, 2], F32, tag="selps")
                    nc.tensor.matmul(out=ps_ie[:], lhsT=gt_m[:, t * P:(t + 1) * P],
                                     rhs=inv[:], start=True, stop=True)
                    w_t = sp.tile([P, 1], F32, tag="xwt")
                    nc.vector.tensor_mul(out=w_t[:], in0=m_ts[t][:, :1].bitcast(F32),
                                         in1=ps_ie[:, :1])
                    ps_xe = pp.tile([P, D], F32, tag="mmps")
                    nc.tensor.matmul(out=ps_xe[:], lhsT=p2t[:, t * P:(t + 1) * P],
                                     rhs=x_in[:], start=True, stop=True)
                    wz = sp.tile([P, D], F32R, tag="xwz")
                    nc.vector.tensor_scalar_mul(out=wz[:], in0=ps_xe[:],
                                                scalar1=w_t[:])
                    nc.tensor.matmul(out=ps_agg[:], lhsT=g_m[:, t, :], rhs=wz[:],
                                     start=(t == 0), stop=(t == 7))
                emb = cp.tile([K, D], F32R, tag="xemb")
                t1 = sp.tile([K, D], F32, tag="xt1")
                nc.vector.tensor_scalar_mul(out=t1[:], in0=ps_agg[:],
                                            scalar1=a_agg[:])
                t2 = sp.tile([K, D], F32, tag="xt2")
                nc.vector.tensor_scalar_mul(out=t2[:], in0=x_in[:].bitcast(F32),
                                            scalar1=a_x[:])
                nc.vector.tensor_add(out=emb[:], in0=t1[:], in1=t2[:])
                embT = transpose_k(emb, tag + "embT")
                ps_hn = mm_kd(embT, lw4, tag + "hnmm", bias=(ones_row, lbr))
                h_next = cp.tile([K, D], F32R, tag=tag + "hnext")
                nc.vector.tensor_scalar_max(out=h_next[:], in0=ps_hn[:], scalar1=0.0)
                return h_next

            def w2_full(w2r, tag):
                ps_w = pp.tile([P, D], F32, tag="mmps")
                nc.tensor.matmul(out=ps_w[:], lhsT=ones_p1[:], rhs=w2r[:],
                                 start=True, stop=True)
                wf = cp.tile([P, D], F32, tag=tag)
                nc.vector.tensor_copy(out=wf[:], in_=ps_w[:])
                return wf

            w1a1 = load_c("w1a_1", [P, 4, D]); w1b1 = load_c("w1b_1", [P, 4, D])
            b11 = load_c("b1_1", [1, D]); w21 = load_c("w2_1", [1, D])
            lw1 = load_c("lw_1", [P, 4, D]); lb1 = load_c("lb_1", [1, D])
            w1a2 = load_c("w1a_2", [P, 4, D]); w1b2 = load_c("w1b_2", [P, 4, D])
            b12 = load_c("b1_2", [1, D]); w22 = load_c("w2_2", [1, D])
            lw2 = load_c("lw_2", [P, 4, D]); lb2 = load_c("lb_2", [1, D])
            wo4 = load_c("wout", [P, 4, P]); bo_r = load_c("bout", [1, P])
            w2f1 = w2_full(w21, "w2f1")
            w2f2 = w2_full(w22, "w2f2")

            h1 = explainer(z_cur, w1a1, w1b1, b11, w2f1, b2_1v, lw1, lb1, "e1")
            h2 = explainer(h1, w1a2, w1b2, b12, w2f2, b2_2v, lw2, lb2, "e2")
            h2T = transpose_k(h2, "h2T")
            ps_out = mm_kd(h2T, wo4, "omm", width=P, bias=(ones_row, bo_r))
            out_sb = sp.tile([K, P], F32, tag="outsb")
            nc.vector.tensor_copy(out=out_sb[:], in_=ps_out[:])
            nc.sync.dma_start(out=out_t[:, :], in_=out_sb[:])
    return nc


# ------------------------------------------------------------- host pipeline

_cache = {}


def kernel(x, edge_index, Wa, ba, Wp, bp,
           e1_W1, e1_b1, e1_W2, e1_b2, e1_lW, e1_lb,
           e2_W1, e2_b1, e2_W2, e2_b2, e2_lW, e2_lb,
           Wout, bout):
    x = np.asarray(x, np.float32)
    edge_index = np.asarray(edge_index)
    row = edge_index[0].astype(np.int64)
    col = edge_index[1].astype(np.int64)

    owner_r = row // SH_REAL
    owner_c = col // SH_REAL
    pos_r = owner_r * SH + (row - owner_r * SH_REAL)
    pos_c = owner_c * SH + (col - owner_c * SH_REAL)
    deg = np.bincount(row, minlength=N).astype(np.float32)

    xp = np.zeros((NCORE, SH, D), np.float32)
    degp = np.zeros((NCORE, SH, 1), np.float32)
    for m in range(NCORE):
        xp[m, :SH_REAL] = x[m * SH_REAL:(m + 1) * SH_REAL]
        degp[m, :SH_REAL, 0] = deg[m * SH_REAL:(m + 1) * SH_REAL]
    wa_in = np.asarray(Wa, np.float32)
    ba_in = np.asarray(ba, np.float32).reshape(1, K)
    if "p1" not in _cache:
        _cache["p1"] = SpmdRunner(build_p1())
    res1 = _cache["p1"]([dict(x=xp[m], wa=wa_in, ba=ba_in, deg=degp[m])
                         for m in range(NCORE)])

    s_full = np.concatenate([res1[m]["s_out"].reshape(SH, K)[:SH_REAL]
                             for m in range(NCORE)], axis=0)
    table = np.concatenate([res1[m]["sb_out"].reshape(SH, 32)
                            for m in range(NCORE)], axis=0)
    table_lines = np.zeros((NLINES, 128), ml_dtypes.bfloat16)
    table_lines[:NPOS // 4] = table.reshape(NPOS // 4, 128)

    cls = ((pos_r & 3) * 4 + (pos_c & 3)).astype(np.int64)
    key = owner_r * 16 + cls
    order = np.argsort(key, kind="stable")
    key_s = key[order]
    pr_s = (pos_r[order] >> 2).astype(np.int16)
    pc_s = (pos_c[order] >> 2).astype(np.int16)
    counts = np.bincount(key_s, minlength=NCORE * 16).reshape(NCORE, 16)
    caps = ((counts.max(axis=0) + 127) // 128) * 128
    caps = np.maximum(caps, 128)
    tot = int(caps.sum())
    pad_idx = np.int16(NPOS // 4)
    ir = np.full((NCORE, tot), pad_idx, np.int16)
    ic = np.full((NCORE, tot), pad_idx, np.int16)
    starts = np.concatenate([[0], np.cumsum(counts.reshape(-1))])[:-1].reshape(NCORE, 16)
    offs = np.concatenate([[0], np.cumsum(caps)])[:-1].astype(np.int64)
    for m in range(NCORE):
        for k in range(16):
            cnt = counts[m, k]
            s0 = starts[m, k]
            ir[m, offs[k]:offs[k] + cnt] = pr_s[s0:s0 + cnt]
            ic[m, offs[k]:offs[k] + cnt] = pc_s[s0:s0 + cnt]
    ir_w = np.ascontiguousarray(
        ir.reshape(NCORE, tot // 16, 16).transpose(0, 2, 1))
    ic_w = np.ascontiguousarray(
        ic.reshape(NCORE, tot // 16, 16).transpose(0, 2, 1))

    caps_t = tuple(int(c) for c in caps)
    ck = ("p2", caps_t)
    if ck not in _cache:
        _cache[ck] = SpmdRunner(build_p2(list(caps_t)))
    res2 = _cache[ck]([dict(tbl=table_lines, idx_r=ir_w[m], idx_c=ic_w[m])
                       for m in range(NCORE)])

    i_of = np.arange(NPT) // 32
    j_of = np.arange(NPT) % 32
    valid = (i_of < K) & (j_of < K)
    p1t = np.zeros((K, NPT), np.float32)
    p1t[np.minimum(i_of, K - 1), np.arange(NPT)] = valid.astype(np.float32)
    p2t = np.zeros((K, NPT), np.float32)
    p2t[np.minimum(j_of, K - 1), np.arange(NPT)] = valid.astype(np.float32)
    g_m = np.zeros((NPT, K), np.float32)
    g_m[np.arange(NPT), np.minimum(i_of, K - 1)] = valid.astype(np.float32)
    gt_m = np.ascontiguousarray(g_m.T)
    p2_m = np.zeros((NPT, K), np.float32)
    p2_m[np.arange(NPT), np.minimum(j_of, K - 1)] = valid.astype(np.float32)

    f32 = lambda a: np.ascontiguousarray(np.asarray(a, np.float32))
    in3 = dict(
        zxp=np.concatenate([res1[m]["zx_out"] for m in range(NCORE)], 0),
        ssp=np.concatenate([res1[m]["ss_out"] for m in range(NCORE)], 0),
        adjp=np.concatenate([res2[m]["adj_out"] for m in range(NCORE)], 0),
        volp=np.concatenate([res1[m]["vol_out"] for m in range(NCORE)], 0),
        wp=f32(Wp), bp=f32(bp).reshape(1, D),
        w1a_1=f32(np.asarray(e1_W1)[:D]), w1b_1=f32(np.asarray(e1_W1)[D:]),
        b1_1=f32(e1_b1).reshape(1, D), w2_1=f32(e1_W2).reshape(1, D),
        lw_1=f32(e1_lW), lb_1=f32(e1_lb).reshape(1, D),
        w1a_2=f32(np.asarray(e2_W1)[:D]), w1b_2=f32(np.asarray(e2_W1)[D:]),
        b1_2=f32(e2_b1).reshape(1, D), w2_2=f32(e2_W2).reshape(1, D),
        lw_2=f32(e2_lW), lb_2=f32(e2_lb).reshape(1, D),
        wout=f32(Wout), bout=f32(bout).reshape(1, P),
        p1t=p1t, p2t=p2t, g_m=g_m, gt_m=gt_m, p2_m=p2_m,
        eye=np.eye(K, dtype=np.float32),
    )
    b2_1v = float(np.asarray(e1_b2).reshape(-1)[0])
    b2_2v = float(np.asarray(e2_b2).reshape(-1)[0])
    pk = ("p3", round(b2_1v, 9), round(b2_2v, 9))
    if pk not in _cache:
        _cache[pk] = SpmdRunner(build_p3(b2_1v, b2_2v))
    res3 = _cache[pk]([in3] * NCORE)
    out = res3[0]["out"]
    mincut = np.float32(res3[0]["mincut"][0, 0])
    ortho = np.float32(res3[0]["ortho"][0, 0])
    Z = res3[0]["z"]
    return (out, mincut, ortho, Z, s_full)
